# revision 4
# baseline (speedup 1.0000x reference)
"""Causal attention kernel for Trainium2, 8 NeuronCores.

Problem: x [4, 4096, 1024] fp32, Wq/Wk/Wv [1024, 1024] fp32.
  q = x @ Wq.T ; k = x @ Wk.T ; v = x @ Wv.T  (per batch)
  out = softmax(causal(q k^T / sqrt(1024))) @ v

Sharding: 8 cores = 4 batches x 2 parities. Core (b, p) computes output rows
{p, p+2, ...} of batch b (interleaved rows -> balanced causal work, and the
diagonal-tile masks are identical for every row-block, so one uniform SPMD
program works for all cores with masks passed as data).

Per-core dataflow (all matmuls fp16 with fp32 PSUM accumulation):
  qT[e, nq] = WqT^T-chunks x xqT        (nq = 2048 local rows)
  kT[e, m]  = WkT-chunks x xT           (m = 4096)
  V[m, e]   = xT-chunks x WvT
  scoresT[m-tile, nq-blk] = kT-chunks^T x qT-chunks   (causal extent only)
  probsT = exp(scoresT) * mask01        (no max subtraction; |scores| <~ 6)
  sums[nq] = probsT^T x ones            (PE ones-matmul, PSUM accumulated)
  ctx[nq, e] = probsT^T x V             (PSUM accumulated over m-tiles)
  out = ctx / sums
"""

import numpy as np

import concourse.bacc as bacc
import concourse.mybir as mybir
from concourse import tile
from concourse.bass_utils import run_bass_kernel_spmd

B, N, D = 4, 4096, 1024
NQ = N // 2          # local rows per core (one parity of one batch)
P = 128              # partitions
NB = NQ // 256       # nq blocks of 256 local rows (8)
DC = D // P          # d chunks (8)
EB = D // P          # e blocks (8)
MT = N // P          # m tiles of 128 (32)
MC = N // 512        # m chunks of 512 (8)

F32 = mybir.dt.float32
F16 = mybir.dt.float16


def build_nc():
    nc = bacc.Bacc(None, target_bir_lowering=False)

    xq_t = nc.declare_dram_parameter("xq_t", [D, NQ], F16, isOutput=False)
    x_t = nc.declare_dram_parameter("x_t", [D, N], F16, isOutput=False)
    wq_t = nc.declare_dram_parameter("wq_t", [D, D], F16, isOutput=False)
    wk_t = nc.declare_dram_parameter("wk_t", [D, D], F16, isOutput=False)
    wv_t = nc.declare_dram_parameter("wv_t", [D, D], F16, isOutput=False)
    mask = nc.declare_dram_parameter("mask", [P, 1024], F16, isOutput=False)
    out = nc.declare_dram_parameter("out", [NQ, D], F32, isOutput=True)

    xq_r = xq_t.rearrange("(a p) q -> p a q", p=P)
    x_r = x_t.rearrange("(a p) m -> p a m", p=P)
    wq_r = wq_t.rearrange("(a p) e -> p a e", p=P)
    wk_r = wk_t.rearrange("(a p) e -> p a e", p=P)
    wv_r = wv_t.rearrange("(a p) e -> p a e", p=P)

    with tile.TileContext(nc) as tc:
        with (
            tc.tile_pool(name="const", bufs=1) as const_pool,
            tc.tile_pool(name="w", bufs=2) as w_pool,
            tc.tile_pool(name="persist", bufs=1) as persist,
            tc.tile_pool(name="stream", bufs=2) as stream,
            tc.tile_pool(name="exp", bufs=6) as exp_pool,
            tc.tile_pool(name="outs", bufs=2) as out_pool,
            tc.tile_pool(name="small", bufs=4) as small_pool,
            tc.tile_pool(name="mm", bufs=2, space="PSUM") as mm_pool,
            tc.tile_pool(name="ctx", bufs=4, space="PSUM") as ctx_pool,
            tc.tile_pool(name="sums", bufs=2, space="PSUM") as sum_pool,
        ):
            masks = const_pool.tile([P, 1024], F16, tag="mask")
            nc.sync.dma_start(out=masks[:], in_=mask[:])
            ones = const_pool.tile([P, 1], F16, tag="ones")
            nc.any.memset(ones[:], 1.0)

            kT = persist.tile([P, EB, N], F16, tag="kT")
            V = persist.tile([P, MT, D], F16, tag="v")

            def q_phase(wq_tile, half):
                """Project one half (1024 local rows) of qT."""
                qT = persist.tile([P, EB, NQ // 2], F16, tag="qT")
                for nqc in range(2):
                    xq = stream.tile([P, DC, 512], F16, tag="x")
                    col0 = half * 1024 + nqc * 512
                    nc.sync.dma_start(out=xq[:], in_=xq_r[:, :, col0 : col0 + 512])
                    for eb in range(EB):
                        ps = mm_pool.tile([P, 512], F32, tag="mm")
                        for dc in range(DC):
                            nc.tensor.matmul(
                                ps[:],
                                wq_tile[:, dc, eb * P : (eb + 1) * P],
                                xq[:, dc, :],
                                start=(dc == 0),
                                stop=(dc == DC - 1),
                            )
                        nc.scalar.copy(qT[:, eb, nqc * 512 : (nqc + 1) * 512], ps[:])
                return qT

            def kv_phase(wk_tile, wv_tile):
                for mc in range(MC):
                    xt = stream.tile([P, DC, 512], F16, tag="x")
                    nc.sync.dma_start(out=xt[:], in_=x_r[:, :, mc * 512 : (mc + 1) * 512])
                    for eb in range(EB):
                        ps = mm_pool.tile([P, 512], F32, tag="mm")
                        for dc in range(DC):
                            nc.tensor.matmul(
                                ps[:],
                                wk_tile[:, dc, eb * P : (eb + 1) * P],
                                xt[:, dc, :],
                                start=(dc == 0),
                                stop=(dc == DC - 1),
                            )
                        nc.scalar.copy(kT[:, eb, mc * 512 : (mc + 1) * 512], ps[:])
                    for i in range(4):
                        mb = 4 * mc + i
                        for eh in range(2):
                            ps = mm_pool.tile([P, 512], F32, tag="mm")
                            for dc in range(DC):
                                nc.tensor.matmul(
                                    ps[:],
                                    xt[:, dc, i * P : (i + 1) * P],
                                    wv_tile[:, dc, eh * 512 : (eh + 1) * 512],
                                    start=(dc == 0),
                                    stop=(dc == DC - 1),
                                )
                            nc.scalar.copy(V[:, mb, eh * 512 : (eh + 1) * 512], ps[:])

            def attn_block(qT, j):
                """Attention for nq block j (256 local rows)."""
                jj = j % 4  # index within the qT half
                ntiles = 4 * j + 4
                ctx = [
                    [
                        ctx_pool.tile([P, 512], F32, tag="ctx", name=f"ctx{su}{eh}")
                        for eh in range(2)
                    ]
                    for su in range(2)
                ]  # [su][eh]
                sums = [
                    sum_pool.tile([P, 1], F32, tag="sums", name=f"sums{su}")
                    for su in range(2)
                ]

                pending = None  # probs tile of previous t, for SW pipelining

                def consume(pt, t):
                    first = t == 0
                    last = t == ntiles - 1
                    for su in range(2):
                        lhsT = pt[:, su * P : (su + 1) * P]
                        nc.tensor.matmul(
                            sums[su][:], lhsT, ones[:], start=first, stop=last
                        )
                        for eh in range(2):
                            nc.tensor.matmul(
                                ctx[su][eh][:],
                                lhsT,
                                V[:, t, eh * 512 : (eh + 1) * 512],
                                start=first,
                                stop=last,
                            )

                for t in range(ntiles):
                    ps = mm_pool.tile([P, 256], F32, tag="mm")
                    for eb in range(EB):
                        nc.tensor.matmul(
                            ps[:],
                            kT[:, eb, t * P : (t + 1) * P],
                            qT[:, eb, jj * 256 : (jj + 1) * 256],
                            start=(eb == 0),
                            stop=(eb == EB - 1),
                        )
                    et = exp_pool.tile([P, 256], F16, tag="et")
                    nc.scalar.activation(et[:], ps[:], mybir.ActivationFunctionType.Exp)
                    s = t - 4 * j
                    if s >= 0:  # diagonal tile: zero out masked entries
                        me = exp_pool.tile([P, 256], F16, tag="et")
                        nc.vector.tensor_mul(
                            me[:], et[:], masks[:, s * 256 : (s + 1) * 256]
                        )
                        pt = me
                    else:
                        pt = et
                    if pending is not None:
                        consume(*pending)
                    pending = (pt, t)
                consume(*pending)

                for su in range(2):
                    recip = small_pool.tile([P, 1], F32, tag="recip")
                    nc.vector.reciprocal(recip[:], sums[su][:])
                    ob = out_pool.tile([P, D], F32, tag="ob")
                    for eh in range(2):
                        nc.vector.tensor_scalar_mul(
                            ob[:, eh * 512 : (eh + 1) * 512], ctx[su][eh][:], recip[:]
                        )
                    r0 = j * 256 + su * P
                    nc.sync.dma_start(out=out[r0 : r0 + P, :], in_=ob[:])

            # ---- phase emission ----
            wq = w_pool.tile([P, DC, D], F16, tag="w")
            nc.sync.dma_start(out=wq[:], in_=wq_r[:])
            wk = w_pool.tile([P, DC, D], F16, tag="w")
            nc.sync.dma_start(out=wk[:], in_=wk_r[:])

            qTA = q_phase(wq, half=0)

            wv = w_pool.tile([P, DC, D], F16, tag="w")
            nc.sync.dma_start(out=wv[:], in_=wv_r[:])
            kv_phase(wk, wv)

            for j in range(4):
                attn_block(qTA, j)
            wq2 = w_pool.tile([P, DC, D], F16, tag="w")
            nc.sync.dma_start(out=wq2[:], in_=wq_r[:])
            qTB = q_phase(wq2, half=1)
            for j in range(4, NB):
                attn_block(qTB, j)

    nc.compile()
    return nc


_NC_CACHE = None


def _get_nc():
    global _NC_CACHE
    if _NC_CACHE is None:
        _NC_CACHE = build_nc()
    return _NC_CACHE


def make_masks():
    """mask01[p][m_l, 256*s + i] = 1 if (128*s + m_l) <= (2*i + p) else 0."""
    i = np.arange(256)
    m_l = np.arange(P)
    out = []
    for p in range(2):
        tiles = []
        for s in range(4):
            allow = (128 * s + m_l[:, None]) <= (2 * i[None, :] + p)
            tiles.append(allow.astype(np.float16))
        out.append(np.concatenate(tiles, axis=1))
    return out


def make_in_maps(x, Wq, Wk, Wv):
    scale = np.float32(1.0 / np.sqrt(D))
    wq_t = np.ascontiguousarray((Wq.T * scale).astype(np.float16))
    wk_t = np.ascontiguousarray(Wk.T.astype(np.float16))
    wv_t = np.ascontiguousarray(Wv.T.astype(np.float16))
    masks = make_masks()
    x16 = x.astype(np.float16)
    in_maps = []
    for c in range(8):
        b, p = c // 2, c % 2
        in_maps.append(
            {
                "xq_t": np.ascontiguousarray(x16[b, p::2, :].T),
                "x_t": np.ascontiguousarray(x16[b].T),
                "wq_t": wq_t,
                "wk_t": wk_t,
                "wv_t": wv_t,
                "mask": masks[p],
            }
        )
    return in_maps


def kernel(x, Wq, Wk, Wv):
    nc = _get_nc()
    in_maps = make_in_maps(x, Wq, Wk, Wv)
    res = run_bass_kernel_spmd(nc, in_maps, core_ids=list(range(8)))
    out = np.empty((B, N, D), dtype=np.float32)
    for c in range(8):
        b, p = c // 2, c % 2
        out[b, p::2, :] = res.results[c]["out"]
    return out


# revision 9
# speedup vs baseline: 1.2800x; 1.2800x over previous
"""Causal attention kernel for Trainium2, 8 NeuronCores.

Problem: x [4, 4096, 1024] fp32, Wq/Wk/Wv [1024, 1024] fp32.
  q = x @ Wq.T ; k = x @ Wk.T ; v = x @ Wv.T  (per batch)
  out = softmax(causal(q k^T / sqrt(1024))) @ v

Sharding: 8 cores = 4 batches x 2 parities. Core (b, p) computes output rows
{p, p+2, ...} of batch b (interleaved rows -> balanced causal work, and the
diagonal-tile masks are identical for every row-block, so one uniform SPMD
program works for all cores with masks passed as data).

Per-core dataflow (all matmuls fp16 with fp32 PSUM accumulation):
  qT[e, nq] = WqT^T-chunks x xqT        (nq = 2048 local rows)
  kT[e, m]  = WkT-chunks x xT           (m = 4096)
  V[m, e]   = xT-chunks x WvT
  scoresT[m-tile, nq-blk] = kT-chunks^T x qT-chunks   (causal extent only)
  probsT = exp(scoresT) * mask01        (no max subtraction; |scores| <~ 6)
  sums[nq] = probsT^T x ones            (PE ones-matmul, PSUM accumulated)
  ctx[nq, e] = probsT^T x V             (PSUM accumulated over m-tiles)
  out = ctx / sums
"""

import numpy as np

import concourse.bacc as bacc
import concourse.mybir as mybir
from concourse import tile

B, N, D = 4, 4096, 1024
NQ = N // 2          # local rows per core (one parity of one batch)
P = 128              # partitions
NB = NQ // 256       # nq blocks of 256 local rows (8)
DC = D // P          # d chunks (8)
EB = D // P          # e blocks (8)
MT = N // P          # m tiles of 128 (32)
MC = N // 512        # m chunks of 512 (8)

F32 = mybir.dt.float32
F16 = mybir.dt.float16


def build_nc():
    nc = bacc.Bacc(None, target_bir_lowering=False)

    xq_t = nc.declare_dram_parameter("xq_t", [D, NQ], F16, isOutput=False)
    x_t = nc.declare_dram_parameter("x_t", [D, N], F16, isOutput=False)
    wq_t = nc.declare_dram_parameter("wq_t", [D, D], F16, isOutput=False)
    wk_t = nc.declare_dram_parameter("wk_t", [D, D], F16, isOutput=False)
    wv_t = nc.declare_dram_parameter("wv_t", [D, D], F16, isOutput=False)
    mask = nc.declare_dram_parameter("mask", [P, 1024], F16, isOutput=False)
    out = nc.declare_dram_parameter("out", [NQ, D], F32, isOutput=True)

    xq_r = xq_t.rearrange("(a p) q -> p a q", p=P)
    x_r = x_t.rearrange("(a p) m -> p a m", p=P)
    wq_r = wq_t.rearrange("(a p) e -> p a e", p=P)
    wk_r = wk_t.rearrange("(a p) e -> p a e", p=P)
    wv_r = wv_t.rearrange("(a p) e -> p a e", p=P)

    with tile.TileContext(nc) as tc:
        with (
            tc.tile_pool(name="const", bufs=1) as const_pool,
            tc.tile_pool(name="w", bufs=2) as w_pool,
            tc.tile_pool(name="persist", bufs=1) as persist,
            tc.tile_pool(name="stream", bufs=2) as stream,
            tc.tile_pool(name="exp", bufs=6) as exp_pool,
            tc.tile_pool(name="outs", bufs=2) as out_pool,
            tc.tile_pool(name="small", bufs=4) as small_pool,
            tc.tile_pool(name="mm", bufs=2, space="PSUM") as mm_pool,
            tc.tile_pool(name="ctx", bufs=4, space="PSUM") as ctx_pool,
            tc.tile_pool(name="sums", bufs=2, space="PSUM") as sum_pool,
        ):
            masks = const_pool.tile([P, 1024], F16, tag="mask")
            nc.sync.dma_start(out=masks[:], in_=mask[:])
            ones = const_pool.tile([P, 1], F16, tag="ones")
            nc.any.memset(ones[:], 1.0)

            kT = persist.tile([P, EB, N], F16, tag="kT")
            V = persist.tile([P, MT, D], F16, tag="v")

            def q_phase(wq_tile, half):
                """Project one half (1024 local rows) of qT."""
                qT = persist.tile([P, EB, NQ // 2], F16, tag="qT")
                for nqc in range(2):
                    xq = stream.tile([P, DC, 512], F16, tag="x")
                    col0 = half * 1024 + nqc * 512
                    nc.sync.dma_start(out=xq[:], in_=xq_r[:, :, col0 : col0 + 512])
                    for eb in range(EB):
                        ps = mm_pool.tile([P, 512], F32, tag="mm")
                        for dc in range(DC):
                            nc.tensor.matmul(
                                ps[:],
                                wq_tile[:, dc, eb * P : (eb + 1) * P],
                                xq[:, dc, :],
                                start=(dc == 0),
                                stop=(dc == DC - 1),
                            )
                        nc.scalar.copy(qT[:, eb, nqc * 512 : (nqc + 1) * 512], ps[:])
                return qT

            def kv_phase(wk_tile, wv_tile):
                for mc in range(MC):
                    xt = stream.tile([P, DC, 512], F16, tag="x")
                    nc.sync.dma_start(out=xt[:], in_=x_r[:, :, mc * 512 : (mc + 1) * 512])
                    for eb in range(EB):
                        ps = mm_pool.tile([P, 512], F32, tag="mm")
                        for dc in range(DC):
                            nc.tensor.matmul(
                                ps[:],
                                wk_tile[:, dc, eb * P : (eb + 1) * P],
                                xt[:, dc, :],
                                start=(dc == 0),
                                stop=(dc == DC - 1),
                            )
                        nc.scalar.copy(kT[:, eb, mc * 512 : (mc + 1) * 512], ps[:])
                    for i in range(4):
                        mb = 4 * mc + i
                        for eh in range(2):
                            ps = mm_pool.tile([P, 512], F32, tag="mm")
                            for dc in range(DC):
                                nc.tensor.matmul(
                                    ps[:],
                                    xt[:, dc, i * P : (i + 1) * P],
                                    wv_tile[:, dc, eh * 512 : (eh + 1) * 512],
                                    start=(dc == 0),
                                    stop=(dc == DC - 1),
                                )
                            nc.scalar.copy(V[:, mb, eh * 512 : (eh + 1) * 512], ps[:])

            def attn_block(qT, j):
                """Attention for nq block j (256 local rows)."""
                jj = j % 4  # index within the qT half
                ntiles = 4 * j + 4
                ctx = [
                    [
                        ctx_pool.tile([P, 512], F32, tag="ctx", name=f"ctx{su}{eh}")
                        for eh in range(2)
                    ]
                    for su in range(2)
                ]  # [su][eh]
                sums = [
                    sum_pool.tile([P, 1], F32, tag="sums", name=f"sums{su}")
                    for su in range(2)
                ]

                pending = None  # probs tile of previous t, for SW pipelining

                def consume(pt, t):
                    first = t == 0
                    last = t == ntiles - 1
                    for su in range(2):
                        lhsT = pt[:, su * P : (su + 1) * P]
                        nc.tensor.matmul(
                            sums[su][:], lhsT, ones[:], start=first, stop=last
                        )
                        for eh in range(2):
                            nc.tensor.matmul(
                                ctx[su][eh][:],
                                lhsT,
                                V[:, t, eh * 512 : (eh + 1) * 512],
                                start=first,
                                stop=last,
                            )

                for t in range(ntiles):
                    ps = mm_pool.tile([P, 256], F32, tag="mm")
                    for eb in range(EB):
                        nc.tensor.matmul(
                            ps[:],
                            kT[:, eb, t * P : (t + 1) * P],
                            qT[:, eb, jj * 256 : (jj + 1) * 256],
                            start=(eb == 0),
                            stop=(eb == EB - 1),
                        )
                    et = exp_pool.tile([P, 256], F16, tag="et")
                    nc.scalar.activation(et[:], ps[:], mybir.ActivationFunctionType.Exp)
                    s = t - 4 * j
                    if s >= 0:  # diagonal tile: zero out masked entries
                        me = exp_pool.tile([P, 256], F16, tag="et")
                        nc.vector.tensor_mul(
                            me[:], et[:], masks[:, s * 256 : (s + 1) * 256]
                        )
                        pt = me
                    else:
                        pt = et
                    if pending is not None:
                        consume(*pending)
                    pending = (pt, t)
                consume(*pending)

                for su in range(2):
                    recip = small_pool.tile([P, 1], F32, tag="recip")
                    nc.vector.reciprocal(recip[:], sums[su][:])
                    ob = out_pool.tile([P, D], F32, tag="ob")
                    for eh in range(2):
                        nc.vector.tensor_scalar_mul(
                            ob[:, eh * 512 : (eh + 1) * 512], ctx[su][eh][:], recip[:]
                        )
                    r0 = j * 256 + su * P
                    nc.sync.dma_start(out=out[r0 : r0 + P, :], in_=ob[:])

            # ---- phase emission ----
            wq = w_pool.tile([P, DC, D], F16, tag="w")
            nc.sync.dma_start(out=wq[:], in_=wq_r[:])
            wk = w_pool.tile([P, DC, D], F16, tag="w")
            nc.sync.dma_start(out=wk[:], in_=wk_r[:])

            qTA = q_phase(wq, half=0)

            wv = w_pool.tile([P, DC, D], F16, tag="w")
            nc.sync.dma_start(out=wv[:], in_=wv_r[:])
            kv_phase(wk, wv)

            for j in range(4):
                attn_block(qTA, j)
            wq2 = w_pool.tile([P, DC, D], F16, tag="w")
            nc.sync.dma_start(out=wq2[:], in_=wq_r[:])
            qTB = q_phase(wq2, half=1)
            for j in range(4, NB):
                attn_block(qTB, j)

    nc.compile()
    return nc


class Runner:
    """Compile once, keep the jitted sharded executable + static inputs on
    device, and rotate donated output buffers across calls.

    Mirrors bass2jax.run_bass_via_pjrt but caches everything reusable.
    `replicated` names inputs identical across cores (shipped once).
    """

    def __init__(self, nc, n_cores=8, replicated=()):
        import jax
        from jax.sharding import Mesh, PartitionSpec, NamedSharding
        from jax.experimental.shard_map import shard_map
        from concourse import bass2jax

        bass2jax.install_neuronx_cc_hook()
        self.jax = jax
        self.nc = nc
        self.n_cores = n_cores
        self.replicated = set(replicated)

        partition_name = (
            nc.partition_id_tensor.name if nc.partition_id_tensor else None
        )
        in_names, out_names, out_avals, zero_outs = [], [], [], []
        for alloc in nc.m.functions[0].allocations:
            if not isinstance(alloc, mybir.MemoryLocationSet):
                continue
            name = alloc.memorylocations[0].name
            if alloc.kind == "ExternalInput":
                if name != partition_name:
                    in_names.append(name)
            elif alloc.kind == "ExternalOutput":
                out_names.append(name)
                shape = tuple(alloc.tensor_shape)
                dtype = mybir.dt.np(alloc.dtype)
                out_avals.append(jax.core.ShapedArray(shape, dtype))
                zero_outs.append(np.zeros((n_cores * shape[0], *shape[1:]), dtype))
        self.in_names, self.out_names, self.out_avals = in_names, out_names, out_avals

        n_params = len(in_names)
        all_names = in_names + out_names
        if partition_name is not None:
            all_names = all_names + [partition_name]
        donate = tuple(range(n_params, n_params + len(out_names)))

        def _body(*args):
            operands = list(args)
            if partition_name is not None:
                operands.append(bass2jax.partition_id_tensor())
            outs = bass2jax._bass_exec_p.bind(
                *operands,
                out_avals=tuple(out_avals),
                in_names=tuple(all_names),
                out_names=tuple(out_names),
                lowering_input_output_aliases=(),
                sim_require_finite=True,
                sim_require_nnan=True,
                nc=nc,
            )
            return tuple(outs)

        devices = jax.devices()[:n_cores]
        self.mesh = Mesh(np.asarray(devices), ("core",))
        self.spec_sharded = NamedSharding(self.mesh, PartitionSpec("core"))
        self.spec_repl = NamedSharding(self.mesh, PartitionSpec())
        in_specs = tuple(
            PartitionSpec() if nm in self.replicated else PartitionSpec("core")
            for nm in in_names
        ) + (PartitionSpec("core"),) * len(out_names)
        out_specs = (PartitionSpec("core"),) * len(out_names)
        self.sharded = jax.jit(
            shard_map(
                _body,
                mesh=self.mesh,
                in_specs=in_specs,
                out_specs=out_specs,
                check_rep=False,
            ),
            donate_argnums=donate,
            keep_unused=True,
        )
        self._spare_outs = zero_outs  # consumed (donated) on first call
        self._static = {}  # name -> device array for cached static inputs

    def put_static(self, name, value):
        """Device-put an input once; reused for every subsequent call."""
        spec = self.spec_repl if name in self.replicated else self.spec_sharded
        self._static[name] = self.jax.device_put(value, spec)

    def __call__(self, in_maps):
        jax = self.jax
        args = []
        for i, nm in enumerate(self.in_names):
            if nm in self._static:
                args.append(self._static[nm])
            elif nm in self.replicated:
                args.append(jax.device_put(in_maps[0][nm], self.spec_repl))
            else:
                cat = np.concatenate([m[nm] for m in in_maps], axis=0)
                args.append(jax.device_put(cat, self.spec_sharded))
        args.extend(self._spare_outs)
        outs = self.sharded(*args)
        outs = [jax.block_until_ready(o) for o in outs]
        # recycle result buffers as the next call's donated outputs (the
        # kernel writes every element, so stale contents are harmless)
        self._spare_outs = list(outs)
        return [
            {
                nm: np.asarray(outs[i]).reshape(
                    self.n_cores, *self.out_avals[i].shape
                )[c]
                for i, nm in enumerate(self.out_names)
            }
            for c in range(self.n_cores)
        ]


_RUNNER = None


def _get_runner():
    global _RUNNER
    if _RUNNER is None:
        nc = build_nc()
        _RUNNER = Runner(nc, n_cores=8, replicated=("wq_t", "wk_t", "wv_t"))
    return _RUNNER


def make_masks():
    """mask01[p][m_l, 256*s + i] = 1 if (128*s + m_l) <= (2*i + p) else 0."""
    i = np.arange(256)
    m_l = np.arange(P)
    out = []
    for p in range(2):
        tiles = []
        for s in range(4):
            allow = (128 * s + m_l[:, None]) <= (2 * i[None, :] + p)
            tiles.append(allow.astype(np.float16))
        out.append(np.concatenate(tiles, axis=1))
    return out


def make_in_maps(x, Wq, Wk, Wv):
    scale = np.float32(1.0 / np.sqrt(D))
    wq_t = np.ascontiguousarray((Wq.T * scale).astype(np.float16))
    wk_t = np.ascontiguousarray(Wk.T.astype(np.float16))
    wv_t = np.ascontiguousarray(Wv.T.astype(np.float16))
    masks = make_masks()
    x16 = x.astype(np.float16)
    in_maps = []
    for c in range(8):
        b, p = c // 2, c % 2
        in_maps.append(
            {
                "xq_t": np.ascontiguousarray(x16[b, p::2, :].T),
                "x_t": np.ascontiguousarray(x16[b].T),
                "wq_t": wq_t,
                "wk_t": wk_t,
                "wv_t": wv_t,
                "mask": masks[p],
            }
        )
    return in_maps


def kernel(x, Wq, Wk, Wv):
    runner = _get_runner()
    in_maps = make_in_maps(x, Wq, Wk, Wv)
    results = runner(in_maps)
    out = np.empty((B, N, D), dtype=np.float32)
    for c in range(8):
        b, p = c // 2, c % 2
        out[b, p::2, :] = results[c]["out"]
    return out


# revision 13
# speedup vs baseline: 5497.5268x; 4294.7925x over previous
"""Causal attention kernel for Trainium2, 8 NeuronCores.

Problem: x [4, 4096, 1024] fp32, Wq/Wk/Wv [1024, 1024] fp32.
  q = x @ Wq.T ; k = x @ Wk.T ; v = x @ Wv.T  (per batch)
  out = softmax(causal(q k^T / sqrt(1024))) @ v

Sharding: 8 cores = 4 batches x 2 parities. Core (b, p) computes output rows
{p, p+2, ...} of batch b (interleaved rows -> balanced causal work, and the
diagonal-tile masks are identical for every row-block, so one uniform SPMD
program works for all cores with masks passed as data).

Per-core dataflow (all matmuls fp16 with fp32 PSUM accumulation):
  qT[e, nq] = WqT^T-chunks x xqT        (nq = 2048 local rows)
  kT[e, m]  = WkT-chunks x xT           (m = 4096)
  V[m, e]   = xT-chunks x WvT
  scoresT[m-tile, nq-blk] = kT-chunks^T x qT-chunks   (causal extent only)
  probsT = exp(scoresT) * mask01        (no max subtraction; |scores| <~ 6)
  sums[nq] = probsT^T x ones            (PE ones-matmul, PSUM accumulated)
  ctx[nq, e] = probsT^T x V             (PSUM accumulated over m-tiles)
  out = ctx / sums
"""

import numpy as np

import concourse.bacc as bacc
import concourse.mybir as mybir
from concourse import tile

B, N, D = 4, 4096, 1024
NQ = N // 2          # local rows per core (one parity of one batch)
P = 128              # partitions
NB = NQ // 256       # nq blocks of 256 local rows (8)
DC = D // P          # d chunks (8)
EB = D // P          # e blocks (8)
MT = N // P          # m tiles of 128 (32)
MC = N // 512        # m chunks of 512 (8)

F32 = mybir.dt.float32
F16 = mybir.dt.float16


def build_nc(n_reps: int = 1):
    """Build the kernel module. n_reps>1 wraps the whole body in an on-device
    For_i loop — used only for benchmarking (wall-clock delta between rep
    counts isolates pure HW execution time from dispatch/transfer overhead).
    """
    nc = bacc.Bacc(None, target_bir_lowering=False)

    xq_t = nc.declare_dram_parameter("xq_t", [D, NQ], F16, isOutput=False)
    x_t = nc.declare_dram_parameter("x_t", [D, N], F16, isOutput=False)
    wq_t = nc.declare_dram_parameter("wq_t", [D, D], F16, isOutput=False)
    wk_t = nc.declare_dram_parameter("wk_t", [D, D], F16, isOutput=False)
    wv_t = nc.declare_dram_parameter("wv_t", [D, D], F16, isOutput=False)
    mask = nc.declare_dram_parameter("mask", [P, 1024], F16, isOutput=False)
    out = nc.declare_dram_parameter("out", [NQ, D], F32, isOutput=True)

    xq_r = xq_t.rearrange("(a p) q -> p a q", p=P)
    x_r = x_t.rearrange("(a p) m -> p a m", p=P)
    wq_r = wq_t.rearrange("(a p) e -> p a e", p=P)
    wk_r = wk_t.rearrange("(a p) e -> p a e", p=P)
    wv_r = wv_t.rearrange("(a p) e -> p a e", p=P)

    with tile.TileContext(nc) as tc:
        with (
            tc.tile_pool(name="const", bufs=1) as const_pool,
            tc.tile_pool(name="w", bufs=2) as w_pool,
            tc.tile_pool(name="persist", bufs=1) as persist,
            tc.tile_pool(name="stream", bufs=2) as stream,
            tc.tile_pool(name="exp", bufs=6) as exp_pool,
            tc.tile_pool(name="outs", bufs=2) as out_pool,
            tc.tile_pool(name="small", bufs=4) as small_pool,
            tc.tile_pool(name="mm", bufs=2, space="PSUM") as mm_pool,
            tc.tile_pool(name="ctx", bufs=4, space="PSUM") as ctx_pool,
            tc.tile_pool(name="sums", bufs=2, space="PSUM") as sum_pool,
        ):
            masks = const_pool.tile([P, 1024], F16, tag="mask")
            nc.sync.dma_start(out=masks[:], in_=mask[:])
            ones = const_pool.tile([P, 1], F16, tag="ones")
            nc.any.memset(ones[:], 1.0)

            kT = persist.tile([P, EB, N], F16, tag="kT")
            V = persist.tile([P, MT, D], F16, tag="v")

            def q_phase(wq_tile, half):
                """Project one half (1024 local rows) of qT."""
                qT = persist.tile([P, EB, NQ // 2], F16, tag="qT")
                for nqc in range(2):
                    xq = stream.tile([P, DC, 512], F16, tag="x")
                    col0 = half * 1024 + nqc * 512
                    nc.sync.dma_start(out=xq[:], in_=xq_r[:, :, col0 : col0 + 512])
                    for eb in range(EB):
                        ps = mm_pool.tile([P, 512], F32, tag="mm")
                        for dc in range(DC):
                            nc.tensor.matmul(
                                ps[:],
                                wq_tile[:, dc, eb * P : (eb + 1) * P],
                                xq[:, dc, :],
                                start=(dc == 0),
                                stop=(dc == DC - 1),
                            )
                        nc.scalar.copy(qT[:, eb, nqc * 512 : (nqc + 1) * 512], ps[:])
                return qT

            def kv_phase(wk_tile, wv_tile):
                for mc in range(MC):
                    xt = stream.tile([P, DC, 512], F16, tag="x")
                    nc.sync.dma_start(out=xt[:], in_=x_r[:, :, mc * 512 : (mc + 1) * 512])
                    for eb in range(EB):
                        ps = mm_pool.tile([P, 512], F32, tag="mm")
                        for dc in range(DC):
                            nc.tensor.matmul(
                                ps[:],
                                wk_tile[:, dc, eb * P : (eb + 1) * P],
                                xt[:, dc, :],
                                start=(dc == 0),
                                stop=(dc == DC - 1),
                            )
                        nc.scalar.copy(kT[:, eb, mc * 512 : (mc + 1) * 512], ps[:])
                    for i in range(4):
                        mb = 4 * mc + i
                        for eh in range(2):
                            ps = mm_pool.tile([P, 512], F32, tag="mm")
                            for dc in range(DC):
                                nc.tensor.matmul(
                                    ps[:],
                                    xt[:, dc, i * P : (i + 1) * P],
                                    wv_tile[:, dc, eh * 512 : (eh + 1) * 512],
                                    start=(dc == 0),
                                    stop=(dc == DC - 1),
                                )
                            nc.scalar.copy(V[:, mb, eh * 512 : (eh + 1) * 512], ps[:])

            def attn_block(qT, j):
                """Attention for nq block j (256 local rows)."""
                jj = j % 4  # index within the qT half
                ntiles = 4 * j + 4
                ctx = [
                    [
                        ctx_pool.tile([P, 512], F32, tag="ctx", name=f"ctx{su}{eh}")
                        for eh in range(2)
                    ]
                    for su in range(2)
                ]  # [su][eh]
                sums = [
                    sum_pool.tile([P, 1], F32, tag="sums", name=f"sums{su}")
                    for su in range(2)
                ]

                pending = None  # probs tile of previous t, for SW pipelining

                def consume(pt, t):
                    first = t == 0
                    last = t == ntiles - 1
                    for su in range(2):
                        lhsT = pt[:, su * P : (su + 1) * P]
                        nc.tensor.matmul(
                            sums[su][:], lhsT, ones[:], start=first, stop=last
                        )
                        for eh in range(2):
                            nc.tensor.matmul(
                                ctx[su][eh][:],
                                lhsT,
                                V[:, t, eh * 512 : (eh + 1) * 512],
                                start=first,
                                stop=last,
                            )

                for t in range(ntiles):
                    ps = mm_pool.tile([P, 256], F32, tag="mm")
                    for eb in range(EB):
                        nc.tensor.matmul(
                            ps[:],
                            kT[:, eb, t * P : (t + 1) * P],
                            qT[:, eb, jj * 256 : (jj + 1) * 256],
                            start=(eb == 0),
                            stop=(eb == EB - 1),
                        )
                    et = exp_pool.tile([P, 256], F16, tag="et")
                    nc.scalar.activation(et[:], ps[:], mybir.ActivationFunctionType.Exp)
                    s = t - 4 * j
                    if s >= 0:  # diagonal tile: zero out masked entries
                        me = exp_pool.tile([P, 256], F16, tag="et")
                        nc.vector.tensor_mul(
                            me[:], et[:], masks[:, s * 256 : (s + 1) * 256]
                        )
                        pt = me
                    else:
                        pt = et
                    if pending is not None:
                        consume(*pending)
                    pending = (pt, t)
                consume(*pending)

                for su in range(2):
                    recip = small_pool.tile([P, 1], F32, tag="recip")
                    nc.vector.reciprocal(recip[:], sums[su][:])
                    ob = out_pool.tile([P, D], F32, tag="ob")
                    for eh in range(2):
                        nc.vector.tensor_scalar_mul(
                            ob[:, eh * 512 : (eh + 1) * 512], ctx[su][eh][:], recip[:]
                        )
                    r0 = j * 256 + su * P
                    nc.sync.dma_start(out=out[r0 : r0 + P, :], in_=ob[:])

            # ---- phase emission ----
            def emit_body():
                wq = w_pool.tile([P, DC, D], F16, tag="w", name="wq")
                nc.sync.dma_start(out=wq[:], in_=wq_r[:])
                wk = w_pool.tile([P, DC, D], F16, tag="w", name="wk")
                nc.sync.dma_start(out=wk[:], in_=wk_r[:])

                qTA = q_phase(wq, half=0)

                wv = w_pool.tile([P, DC, D], F16, tag="w", name="wv")
                nc.sync.dma_start(out=wv[:], in_=wv_r[:])
                kv_phase(wk, wv)

                for j in range(4):
                    attn_block(qTA, j)
                wq2 = w_pool.tile([P, DC, D], F16, tag="w", name="wq2")
                nc.sync.dma_start(out=wq2[:], in_=wq_r[:])
                qTB = q_phase(wq2, half=1)
                for j in range(4, NB):
                    attn_block(qTB, j)

            if n_reps == 1:
                emit_body()
            else:
                with tc.For_i(0, n_reps, 1):
                    emit_body()

    nc.compile()
    return nc


class Runner:
    """Compile once, keep the jitted sharded executable + static inputs on
    device, and rotate donated output buffers across calls.

    Mirrors bass2jax.run_bass_via_pjrt but caches everything reusable.
    `replicated` names inputs identical across cores (shipped once).
    """

    def __init__(self, nc, n_cores=8, replicated=()):
        import jax
        from jax.sharding import Mesh, PartitionSpec, NamedSharding
        from jax.experimental.shard_map import shard_map
        from concourse import bass2jax

        bass2jax.install_neuronx_cc_hook()
        self.jax = jax
        self.nc = nc
        self.n_cores = n_cores
        self.replicated = set(replicated)

        partition_name = (
            nc.partition_id_tensor.name if nc.partition_id_tensor else None
        )
        in_names, out_names, out_avals, zero_outs = [], [], [], []
        for alloc in nc.m.functions[0].allocations:
            if not isinstance(alloc, mybir.MemoryLocationSet):
                continue
            name = alloc.memorylocations[0].name
            if alloc.kind == "ExternalInput":
                if name != partition_name:
                    in_names.append(name)
            elif alloc.kind == "ExternalOutput":
                out_names.append(name)
                shape = tuple(alloc.tensor_shape)
                dtype = mybir.dt.np(alloc.dtype)
                out_avals.append(jax.core.ShapedArray(shape, dtype))
                zero_outs.append(np.zeros((n_cores * shape[0], *shape[1:]), dtype))
        self.in_names, self.out_names, self.out_avals = in_names, out_names, out_avals

        n_params = len(in_names)
        all_names = in_names + out_names
        if partition_name is not None:
            all_names = all_names + [partition_name]
        donate = tuple(range(n_params, n_params + len(out_names)))

        def _body(*args):
            operands = list(args)
            if partition_name is not None:
                operands.append(bass2jax.partition_id_tensor())
            outs = bass2jax._bass_exec_p.bind(
                *operands,
                out_avals=tuple(out_avals),
                in_names=tuple(all_names),
                out_names=tuple(out_names),
                lowering_input_output_aliases=(),
                sim_require_finite=True,
                sim_require_nnan=True,
                nc=nc,
            )
            return tuple(outs)

        devices = jax.devices()[:n_cores]
        self.mesh = Mesh(np.asarray(devices), ("core",))
        self.spec_sharded = NamedSharding(self.mesh, PartitionSpec("core"))
        self.spec_repl = NamedSharding(self.mesh, PartitionSpec())
        in_specs = tuple(
            PartitionSpec() if nm in self.replicated else PartitionSpec("core")
            for nm in in_names
        ) + (PartitionSpec("core"),) * len(out_names)
        out_specs = (PartitionSpec("core"),) * len(out_names)
        self.sharded = jax.jit(
            shard_map(
                _body,
                mesh=self.mesh,
                in_specs=in_specs,
                out_specs=out_specs,
                check_rep=False,
            ),
            donate_argnums=donate,
            keep_unused=True,
        )
        self._spare_outs = zero_outs  # consumed (donated) on first call
        self._static = {}  # name -> device array for cached static inputs

    def put_static(self, name, value):
        """Device-put an input once; reused for every subsequent call."""
        spec = self.spec_repl if name in self.replicated else self.spec_sharded
        self._static[name] = self.jax.device_put(value, spec)

    def __call__(self, in_maps):
        jax = self.jax
        args = []
        for i, nm in enumerate(self.in_names):
            if nm in self._static:
                args.append(self._static[nm])
            elif nm in self.replicated:
                args.append(jax.device_put(in_maps[0][nm], self.spec_repl))
            else:
                cat = np.concatenate([m[nm] for m in in_maps], axis=0)
                args.append(jax.device_put(cat, self.spec_sharded))
        args.extend(self._spare_outs)
        outs = self.sharded(*args)
        outs = [jax.block_until_ready(o) for o in outs]
        # recycle result buffers as the next call's donated outputs (the
        # kernel writes every element, so stale contents are harmless)
        self._spare_outs = list(outs)
        return [
            {
                nm: np.asarray(outs[i]).reshape(
                    self.n_cores, *self.out_avals[i].shape
                )[c]
                for i, nm in enumerate(self.out_names)
            }
            for c in range(self.n_cores)
        ]


_RUNNERS = {}


def _get_runner(n_reps: int = 1):
    if n_reps not in _RUNNERS:
        nc = build_nc(n_reps)
        _RUNNERS[n_reps] = Runner(
            nc, n_cores=8, replicated=("wq_t", "wk_t", "wv_t")
        )
    return _RUNNERS[n_reps]


def make_masks():
    """mask01[p][m_l, 256*s + i] = 1 if (128*s + m_l) <= (2*i + p) else 0."""
    i = np.arange(256)
    m_l = np.arange(P)
    out = []
    for p in range(2):
        tiles = []
        for s in range(4):
            allow = (128 * s + m_l[:, None]) <= (2 * i[None, :] + p)
            tiles.append(allow.astype(np.float16))
        out.append(np.concatenate(tiles, axis=1))
    return out


def make_in_maps(x, Wq, Wk, Wv):
    scale = np.float32(1.0 / np.sqrt(D))
    wq_t = np.ascontiguousarray((Wq.T * scale).astype(np.float16))
    wk_t = np.ascontiguousarray(Wk.T.astype(np.float16))
    wv_t = np.ascontiguousarray(Wv.T.astype(np.float16))
    masks = make_masks()
    x16 = x.astype(np.float16)
    in_maps = []
    for c in range(8):
        b, p = c // 2, c % 2
        in_maps.append(
            {
                "xq_t": np.ascontiguousarray(x16[b, p::2, :].T),
                "x_t": np.ascontiguousarray(x16[b].T),
                "wq_t": wq_t,
                "wk_t": wk_t,
                "wv_t": wv_t,
                "mask": masks[p],
            }
        )
    return in_maps


def kernel(x, Wq, Wk, Wv):
    runner = _get_runner()
    in_maps = make_in_maps(x, Wq, Wk, Wv)
    results = runner(in_maps)
    out = np.empty((B, N, D), dtype=np.float32)
    for c in range(8):
        b, p = c // 2, c % 2
        out[b, p::2, :] = results[c]["out"]
    return out


# revision 36
# speedup vs baseline: 6711.6710x; 1.2209x over previous
"""Causal attention kernel for Trainium2, 8 NeuronCores.

Problem: x [4, 4096, 1024] fp32, Wq/Wk/Wv [1024, 1024] fp32.
  q = x @ Wq.T ; k = x @ Wk.T ; v = x @ Wv.T  (per batch)
  out = softmax(causal(q k^T / sqrt(1024))) @ v

Sharding: 8 cores = 4 batches x 2 parities. Core (b, p) computes output rows
{p, p+2, ...} of batch b (interleaved rows -> balanced causal work, and the
diagonal-tile masks are identical for every row-block, so one uniform SPMD
program works for all cores with masks passed as data).

Per-core dataflow (all matmuls fp16 with fp32 PSUM accumulation):
  qT[e, nq] = WqT-chunks^T x xqT         (nq = 2048 local rows)
  kT[e, m]  = WkT-chunks^T x xT          (m = 4096, SBUF resident)
  V[m, e]   = xT-chunks^T x WvT          (spilled to DRAM, streamed back)
  scoresT[m-tile, nq-blk] = kT-chunks^T x qT-chunks  (causal extent only)
  probsT = exp(scoresT) * mask01         (no max subtraction; |scores| <~ 6)
  sums[nq] = probsT^T x ones             (PE matmul, PSUM accumulated)
  ctx[nq, e] = probsT^T x V              (PSUM accumulated over m-tiles)
  out = ctx / sums

Matmul efficiency: a LDWEIGHTS is emitted per matmul and is only partially
hidden, so projection loops keep one stationary operand loaded and fan out
over several PSUM banks (measured 216 ns/MM at N=512 vs 301 ns without).
"""

import numpy as np

import concourse.bacc as bacc
import concourse.mybir as mybir
from concourse import tile

B, N, D = 4, 4096, 1024
NQ = N // 2          # local rows per core (one parity of one batch)
P = 128              # partitions
NB = NQ // 256       # nq blocks of 256 local rows (8)
DC = D // P          # d chunks (8)
EB = D // P          # e blocks (8)
MT = N // P          # m tiles of 128 (32)

F32 = mybir.dt.float32
F16 = mybir.dt.float16


def build_nc(n_reps: int = 1, phases: str = "all"):
    """Build the kernel module. n_reps>1 wraps the whole body in an on-device
    For_i loop — used only for benchmarking (wall-clock delta between rep
    counts isolates pure HW execution time from dispatch/transfer overhead).
    phases: "all" | "proj" (projections only) | "noctx" (skip context matmuls)
    — benchmark-only variants for phase attribution.
    """
    nc = bacc.Bacc(None, target_bir_lowering=False)

    xq_t = nc.declare_dram_parameter("xq_t", [D, NQ], F16, isOutput=False)
    x_t = nc.declare_dram_parameter("x_t", [D, N], F16, isOutput=False)
    wq_t = nc.declare_dram_parameter("wq_t", [D, D], F16, isOutput=False)
    wk_t = nc.declare_dram_parameter("wk_t", [D, D], F16, isOutput=False)
    wv_t = nc.declare_dram_parameter("wv_t", [D, D], F16, isOutput=False)
    mask = nc.declare_dram_parameter("mask", [P, 1024], F16, isOutput=False)
    out = nc.declare_dram_parameter("out", [NQ, D], F32, isOutput=True)

    xq_r = xq_t.rearrange("(a p) q -> p a q", p=P)
    x_r = x_t.rearrange("(a p) m -> p a m", p=P)
    wq_r = wq_t.rearrange("(a p) e -> p a e", p=P)
    wk_r = wk_t.rearrange("(a p) e -> p a e", p=P)
    wv_r = wv_t.rearrange("(a p) e -> p a e", p=P)

    with tile.TileContext(nc) as tc:
        with (
            tc.tile_pool(name="const", bufs=1) as const_pool,
            tc.tile_pool(name="w", bufs=2) as w_pool,
            tc.tile_pool(name="persist", bufs=1) as persist,
            tc.tile_pool(name="stream", bufs=2) as stream,
            tc.tile_pool(name="vstream", bufs=6) as vstream,
            tc.tile_pool(name="exp", bufs=6) as exp_pool,
            tc.tile_pool(name="outs", bufs=2) as out_pool,
            tc.tile_pool(name="small", bufs=4) as small_pool,
            tc.tile_pool(name="vdram", bufs=1, space="DRAM") as vdram_pool,
            tc.tile_pool(name="panel", bufs=3, space="PSUM") as panel_pool,
            tc.tile_pool(name="ctx", bufs=4, space="PSUM") as ctx_pool,
            tc.tile_pool(name="sums", bufs=1, space="PSUM") as sum_pool,
        ):
            masks = const_pool.tile([P, 1024], F16, tag="mask")
            nc.sync.dma_start(out=masks[:], in_=mask[:])
            ones = const_pool.tile([P, 1], F16, tag="ones")
            nc.any.memset(ones[:], 1.0)

            kT = persist.tile([P, EB, N], F16, tag="kT")
            vdram = vdram_pool.tile([P, MT, D], F16, tag="vdram")

            def q_phase(wq_tile, half):
                """Project one half (1024 local rows) of qT.

                Two e-blocks at a time fan out over 4 PSUM banks (borrowed
                from the ctx pool, idle during projections) so consecutive
                matmuls never hit the same bank (same-bank accumulation
                serializes the PE at ~+60ns/MM).
                """
                qT = persist.tile([P, EB, NQ // 2], F16, tag="qT")
                xq = stream.tile([P, DC, 1024], F16, tag="x", name=f"xq{half}")
                nc.sync.dma_start(
                    out=xq[:], in_=xq_r[:, :, half * 1024 : (half + 1) * 1024]
                )
                for ebp in range(EB // 2):
                    pss = [
                        ctx_pool.tile([P, 512], F32, tag="ctx", name=f"qp{c}")
                        for c in range(4)
                    ]  # c = 2*e + nqc for e in (0,1)
                    for dc in range(DC):
                        for e in range(2):
                            eb = 2 * ebp + e
                            lhsT = wq_tile[:, dc, eb * P : (eb + 1) * P]
                            for nqc in range(2):
                                nc.tensor.matmul(
                                    pss[2 * e + nqc][:],
                                    lhsT,
                                    xq[:, dc, nqc * 512 : (nqc + 1) * 512],
                                    start=(dc == 0),
                                    stop=(dc == DC - 1),
                                )
                    for e in range(2):
                        eb = 2 * ebp + e
                        for nqc in range(2):
                            nc.scalar.copy(
                                qT[:, eb, nqc * 512 : (nqc + 1) * 512],
                                pss[2 * e + nqc][:],
                            )
                return qT

            def kv_phase(wk_tile, wv_tile):
                """kT and V projections, streaming x_t in 2048-col halves.

                Each stationary operand fans out over 4 PSUM banks (borrowed
                from the ctx pool, idle during projections) — same-bank
                accumulation chains serialize the PE. V is spilled to DRAM
                and streamed back during attention (frees 64KB/partition of
                SBUF for larger x chunks).
                """
                for mh in range(2):
                    xt = stream.tile([P, DC, 2048], F16, tag="x", name=f"xt{mh}")
                    nc.sync.dma_start(
                        out=xt[:], in_=x_r[:, :, mh * 2048 : (mh + 1) * 2048]
                    )
                    for eb in range(EB):
                        pss = [
                            ctx_pool.tile([P, 512], F32, tag="ctx", name=f"kp{c}")
                            for c in range(4)
                        ]
                        for dc in range(DC):
                            lhsT = wk_tile[:, dc, eb * P : (eb + 1) * P]
                            for mc in range(4):
                                nc.tensor.matmul(
                                    pss[mc][:],
                                    lhsT,
                                    xt[:, dc, mc * 512 : (mc + 1) * 512],
                                    start=(dc == 0),
                                    stop=(dc == DC - 1),
                                )
                        for mc in range(4):
                            col0 = mh * 2048 + mc * 512
                            nc.scalar.copy(kT[:, eb, col0 : col0 + 512], pss[mc][:])
                    for ip in range(8):  # pairs of m blocks within this half
                        pss = [
                            ctx_pool.tile([P, 512], F32, tag="ctx", name=f"vp{c}")
                            for c in range(4)
                        ]  # c = 2*i + eh
                        for dc in range(DC):
                            for i2 in range(2):
                                i = 2 * ip + i2
                                lhsT = xt[:, dc, i * P : (i + 1) * P]
                                for eh in range(2):
                                    nc.tensor.matmul(
                                        pss[2 * i2 + eh][:],
                                        lhsT,
                                        wv_tile[:, dc, eh * 512 : (eh + 1) * 512],
                                        start=(dc == 0),
                                        stop=(dc == DC - 1),
                                    )
                        for i2 in range(2):
                            mb = mh * 16 + 2 * ip + i2
                            vs = vstream.tile([P, D], F16, tag="vout", bufs=3)
                            for eh in range(2):
                                nc.scalar.copy(
                                    vs[:, eh * 512 : (eh + 1) * 512],
                                    pss[2 * i2 + eh][:],
                                )
                            nc.sync.dma_start(out=vdram[:, mb, :], in_=vs[:])

            def attn_block(qT, j):
                """Attention for nq block j (256 local rows)."""
                jj = j % 4  # index within the qT half
                ntiles = 4 * j + 4
                ctx = [
                    [
                        ctx_pool.tile([P, 512], F32, tag="ctx", name=f"ctx{su}{eh}")
                        for eh in range(2)
                    ]
                    for su in range(2)
                ]  # [su][eh]
                sums = sum_pool.tile([P, 2], F32, tag="sums", name=f"sums{j}")

                pending = None  # (probs tile, V tile, t) for SW pipelining

                def consume(pt, vt, t):
                    first = t == 0
                    last = t == ntiles - 1
                    for su in range(2):
                        lhsT = pt[:, su * P : (su + 1) * P]
                        # su=0's start=True zeroes the whole bank (both
                        # columns); su=1 must not re-start or it would wipe
                        # su=0's accumulator. Its first write lands on
                        # cleared has_written bits, which overwrites.
                        nc.tensor.matmul(
                            sums[:, su : su + 1], lhsT, ones[:],
                            start=(first and su == 0), stop=last,
                            skip_group_check=True,
                        )
                        if phases == "noctx":
                            continue
                        for eh in range(2):
                            nc.tensor.matmul(
                                ctx[su][eh][:],
                                lhsT,
                                vt[:, eh * 512 : (eh + 1) * 512],
                                start=first,
                                stop=last,
                            )

                # scores are computed for pairs of m-tiles so consecutive
                # matmuls alternate PSUM banks (same-bank chains serialize)
                for u in range(ntiles // 2):
                    pss = [
                        panel_pool.tile([P, 256], F32, tag="panel", name=f"sc{c}")
                        for c in range(2)
                    ]
                    for eb in range(EB):
                        for c in range(2):
                            t = 2 * u + c
                            nc.tensor.matmul(
                                pss[c][:],
                                kT[:, eb, t * P : (t + 1) * P],
                                qT[:, eb, jj * 256 : (jj + 1) * 256],
                                start=(eb == 0),
                                stop=(eb == EB - 1),
                            )
                    for c in range(2):
                        t = 2 * u + c
                        et = exp_pool.tile([P, 256], F16, tag="et")
                        nc.scalar.activation(
                            et[:], pss[c][:], mybir.ActivationFunctionType.Exp
                        )
                        s = t - 4 * j
                        if s >= 0:  # diagonal tile: zero out masked entries
                            me = exp_pool.tile([P, 256], F16, tag="et")
                            nc.vector.tensor_mul(
                                me[:], et[:], masks[:, s * 256 : (s + 1) * 256]
                            )
                            pt = me
                        else:
                            pt = et
                        vt = vstream.tile([P, D], F16, tag="vin", bufs=6)
                        nc.sync.dma_start(out=vt[:], in_=vdram[:, t, :])
                        if pending is not None:
                            consume(*pending)
                        pending = (pt, vt, t)
                consume(*pending)

                recip = small_pool.tile([P, 2], F32, tag="recip")
                nc.vector.reciprocal(recip[:], sums[:])
                for su in range(2):
                    r0 = j * 256 + su * P
                    if phases == "noctx":
                        nc.sync.dma_start(
                            out=out[r0 : r0 + P, 0:1], in_=recip[:, su : su + 1]
                        )
                        continue
                    ob = out_pool.tile([P, D], F32, tag="ob")
                    for eh in range(2):
                        nc.vector.tensor_scalar_mul(
                            ob[:, eh * 512 : (eh + 1) * 512],
                            ctx[su][eh][:],
                            recip[:, su : su + 1],
                        )
                    nc.sync.dma_start(out=out[r0 : r0 + P, :], in_=ob[:])

            # ---- phase emission ----
            def emit_body():
                wq = w_pool.tile([P, DC, D], F16, tag="w", name="wq")
                nc.sync.dma_start(out=wq[:], in_=wq_r[:])
                wk = w_pool.tile([P, DC, D], F16, tag="w", name="wk")
                nc.sync.dma_start(out=wk[:], in_=wk_r[:])

                qTA = q_phase(wq, half=0)

                wv = w_pool.tile([P, DC, D], F16, tag="w", name="wv")
                nc.sync.dma_start(out=wv[:], in_=wv_r[:])
                kv_phase(wk, wv)

                if phases == "proj":
                    # keep the projection outputs live with token reads
                    ob = out_pool.tile([P, D], F32, tag="ob")
                    nc.scalar.copy(ob[:, 0:512], kT[:, 0, 0:512])
                    vs = vstream.tile([P, D], F16, tag="vin")
                    nc.sync.dma_start(out=vs[:], in_=vdram[:, 0, :])
                    nc.scalar.copy(ob[:, 512:1024], vs[:, 0:512])
                    nc.scalar.copy(ob[:, 0:512], qTA[:, 0, 0:512])
                    nc.sync.dma_start(out=out[0:P, :], in_=ob[:])
                    return

                for j in range(4):
                    attn_block(qTA, j)
                wq2 = w_pool.tile([P, DC, D], F16, tag="w", name="wq2")
                nc.sync.dma_start(out=wq2[:], in_=wq_r[:])
                qTB = q_phase(wq2, half=1)
                for j in range(4, NB):
                    attn_block(qTB, j)

            if n_reps == 1:
                emit_body()
            else:
                with tc.For_i(0, n_reps, 1):
                    emit_body()

    nc.compile()
    return nc


class Runner:
    """Compile once, keep the jitted sharded executable + static inputs on
    device, and rotate donated output buffers across calls.

    Mirrors bass2jax.run_bass_via_pjrt but caches everything reusable.
    `replicated` names inputs identical across cores (shipped once).
    """

    def __init__(self, nc, n_cores=8, replicated=()):
        import jax
        from jax.sharding import Mesh, PartitionSpec, NamedSharding
        from jax.experimental.shard_map import shard_map
        from concourse import bass2jax

        bass2jax.install_neuronx_cc_hook()
        self.jax = jax
        self.nc = nc
        self.n_cores = n_cores
        self.replicated = set(replicated)

        partition_name = (
            nc.partition_id_tensor.name if nc.partition_id_tensor else None
        )
        in_names, out_names, out_avals, zero_outs = [], [], [], []
        for alloc in nc.m.functions[0].allocations:
            if not isinstance(alloc, mybir.MemoryLocationSet):
                continue
            name = alloc.memorylocations[0].name
            if alloc.kind == "ExternalInput":
                if name != partition_name:
                    in_names.append(name)
            elif alloc.kind == "ExternalOutput":
                out_names.append(name)
                shape = tuple(alloc.tensor_shape)
                dtype = mybir.dt.np(alloc.dtype)
                out_avals.append(jax.core.ShapedArray(shape, dtype))
                zero_outs.append(np.zeros((n_cores * shape[0], *shape[1:]), dtype))
        self.in_names, self.out_names, self.out_avals = in_names, out_names, out_avals

        n_params = len(in_names)
        all_names = in_names + out_names
        if partition_name is not None:
            all_names = all_names + [partition_name]
        donate = tuple(range(n_params, n_params + len(out_names)))

        def _body(*args):
            operands = list(args)
            if partition_name is not None:
                operands.append(bass2jax.partition_id_tensor())
            outs = bass2jax._bass_exec_p.bind(
                *operands,
                out_avals=tuple(out_avals),
                in_names=tuple(all_names),
                out_names=tuple(out_names),
                lowering_input_output_aliases=(),
                sim_require_finite=True,
                sim_require_nnan=True,
                nc=nc,
            )
            return tuple(outs)

        devices = jax.devices()[:n_cores]
        self.mesh = Mesh(np.asarray(devices), ("core",))
        self.spec_sharded = NamedSharding(self.mesh, PartitionSpec("core"))
        self.spec_repl = NamedSharding(self.mesh, PartitionSpec())
        in_specs = tuple(
            PartitionSpec() if nm in self.replicated else PartitionSpec("core")
            for nm in in_names
        ) + (PartitionSpec("core"),) * len(out_names)
        out_specs = (PartitionSpec("core"),) * len(out_names)
        self.sharded = jax.jit(
            shard_map(
                _body,
                mesh=self.mesh,
                in_specs=in_specs,
                out_specs=out_specs,
                check_rep=False,
            ),
            donate_argnums=donate,
            keep_unused=True,
        )
        self._spare_outs = zero_outs  # consumed (donated) on first call
        self._static = {}  # name -> device array for cached static inputs

    def put_static(self, name, value):
        """Device-put an input once; reused for every subsequent call."""
        spec = self.spec_repl if name in self.replicated else self.spec_sharded
        self._static[name] = self.jax.device_put(value, spec)

    def __call__(self, in_maps, fetch=True):
        jax = self.jax
        args = []
        for i, nm in enumerate(self.in_names):
            if nm in self._static:
                args.append(self._static[nm])
            elif nm in self.replicated:
                args.append(jax.device_put(in_maps[0][nm], self.spec_repl))
            else:
                cat = np.concatenate([m[nm] for m in in_maps], axis=0)
                args.append(jax.device_put(cat, self.spec_sharded))
        args.extend(self._spare_outs)
        outs = self.sharded(*args)
        outs = [jax.block_until_ready(o) for o in outs]
        if not fetch:
            # benchmark mode: leave results on device (D2H over the axon
            # tunnel is slow and jittery); recycle buffers for donation
            self._spare_outs = list(outs)
            return None
        # recycle result buffers as the next call's donated outputs (the
        # kernel writes every element, so stale contents are harmless)
        self._spare_outs = list(outs)
        return [
            {
                nm: np.asarray(outs[i]).reshape(
                    self.n_cores, *self.out_avals[i].shape
                )[c]
                for i, nm in enumerate(self.out_names)
            }
            for c in range(self.n_cores)
        ]


_RUNNERS = {}


def _get_runner(n_reps: int = 1, phases: str = "all"):
    key = (n_reps, phases)
    if key not in _RUNNERS:
        nc = build_nc(n_reps, phases)
        _RUNNERS[key] = Runner(nc, n_cores=8, replicated=("wq_t", "wk_t", "wv_t"))
    return _RUNNERS[key]


def make_masks():
    """mask01[p][m_l, 256*s + i] = 1 if (128*s + m_l) <= (2*i + p) else 0."""
    i = np.arange(256)
    m_l = np.arange(P)
    out = []
    for p in range(2):
        tiles = []
        for s in range(4):
            allow = (128 * s + m_l[:, None]) <= (2 * i[None, :] + p)
            tiles.append(allow.astype(np.float16))
        out.append(np.concatenate(tiles, axis=1))
    return out


def make_in_maps(x, Wq, Wk, Wv):
    scale = np.float32(1.0 / np.sqrt(D))
    wq_t = np.ascontiguousarray((Wq.T * scale).astype(np.float16))
    wk_t = np.ascontiguousarray(Wk.T.astype(np.float16))
    wv_t = np.ascontiguousarray(Wv.T.astype(np.float16))
    masks = make_masks()
    x16 = x.astype(np.float16)
    in_maps = []
    for c in range(8):
        b, p = c // 2, c % 2
        in_maps.append(
            {
                "xq_t": np.ascontiguousarray(x16[b, p::2, :].T),
                "x_t": np.ascontiguousarray(x16[b].T),
                "wq_t": wq_t,
                "wk_t": wk_t,
                "wv_t": wv_t,
                "mask": masks[p],
            }
        )
    return in_maps


def kernel(x, Wq, Wk, Wv):
    runner = _get_runner()
    in_maps = make_in_maps(x, Wq, Wk, Wv)
    results = runner(in_maps)
    out = np.empty((B, N, D), dtype=np.float32)
    for c in range(8):
        b, p = c // 2, c % 2
        out[b, p::2, :] = results[c]["out"]
    return out


# revision 39
# speedup vs baseline: 6744.4461x; 1.0049x over previous
"""Causal attention kernel for Trainium2, 8 NeuronCores.

Problem: x [4, 4096, 1024] fp32, Wq/Wk/Wv [1024, 1024] fp32.
  q = x @ Wq.T ; k = x @ Wk.T ; v = x @ Wv.T  (per batch)
  out = softmax(causal(q k^T / sqrt(1024))) @ v

Sharding: 8 cores = 4 batches x 2 parities. Core (b, p) computes output rows
{p, p+2, ...} of batch b (interleaved rows -> balanced causal work, and the
diagonal-tile masks are identical for every row-block, so one uniform SPMD
program works for all cores with masks passed as data).

Per-core dataflow (all matmuls fp16 with fp32 PSUM accumulation):
  qT[e, nq] = WqT-chunks^T x xqT         (nq = 2048 local rows)
  kT[e, m]  = WkT-chunks^T x xT          (m = 4096, SBUF resident)
  V[m, e]   = xT-chunks^T x WvT          (spilled to DRAM, streamed back)
  scoresT[m-tile, nq-blk] = kT-chunks^T x qT-chunks  (causal extent only)
  probsT = exp(scoresT) * mask01         (no max subtraction; |scores| <~ 6)
  sums[nq] = probsT^T x ones             (PE matmul, PSUM accumulated)
  ctx[nq, e] = probsT^T x V              (PSUM accumulated over m-tiles)
  out = ctx / sums

Matmul efficiency: a LDWEIGHTS is emitted per matmul and is only partially
hidden, so projection loops keep one stationary operand loaded and fan out
over several PSUM banks (measured 216 ns/MM at N=512 vs 301 ns without).
"""

import numpy as np

import concourse.bacc as bacc
import concourse.mybir as mybir
from concourse import tile

B, N, D = 4, 4096, 1024
NQ = N // 2          # local rows per core (one parity of one batch)
P = 128              # partitions
NB = NQ // 256       # nq blocks of 256 local rows (8)
DC = D // P          # d chunks (8)
EB = D // P          # e blocks (8)
MT = N // P          # m tiles of 128 (32)

F32 = mybir.dt.float32
F16 = mybir.dt.float16


def build_nc(n_reps: int = 1, phases: str = "all"):
    """Build the kernel module. n_reps>1 wraps the whole body in an on-device
    For_i loop — used only for benchmarking (wall-clock delta between rep
    counts isolates pure HW execution time from dispatch/transfer overhead).
    phases: "all" | "proj" (projections only) | "noctx" (skip context matmuls)
    — benchmark-only variants for phase attribution.
    """
    nc = bacc.Bacc(None, target_bir_lowering=False)

    xq_t = nc.declare_dram_parameter("xq_t", [D, NQ], F16, isOutput=False)
    x_t = nc.declare_dram_parameter("x_t", [D, N], F16, isOutput=False)
    wq_t = nc.declare_dram_parameter("wq_t", [D, D], F16, isOutput=False)
    wk_t = nc.declare_dram_parameter("wk_t", [D, D], F16, isOutput=False)
    wv_t = nc.declare_dram_parameter("wv_t", [D, D], F16, isOutput=False)
    mask = nc.declare_dram_parameter("mask", [P, 1024], F16, isOutput=False)
    out = nc.declare_dram_parameter("out", [NQ, D], F32, isOutput=True)

    xq_r = xq_t.rearrange("(a p) q -> p a q", p=P)
    x_r = x_t.rearrange("(a p) m -> p a m", p=P)
    wq_r = wq_t.rearrange("(a p) e -> p a e", p=P)
    wk_r = wk_t.rearrange("(a p) e -> p a e", p=P)
    wv_r = wv_t.rearrange("(a p) e -> p a e", p=P)

    with tile.TileContext(nc) as tc:
        with (
            tc.tile_pool(name="const", bufs=1) as const_pool,
            tc.tile_pool(name="w", bufs=2) as w_pool,
            tc.tile_pool(name="persist", bufs=1) as persist,
            tc.tile_pool(name="stream", bufs=2) as stream,
            tc.tile_pool(name="vstream", bufs=5) as vstream,
            tc.tile_pool(name="exp", bufs=8) as exp_pool,
            tc.tile_pool(name="outs", bufs=2) as out_pool,
            tc.tile_pool(name="small", bufs=4) as small_pool,
            tc.tile_pool(name="vdram", bufs=1, space="DRAM") as vdram_pool,
            tc.tile_pool(name="panel", bufs=3, space="PSUM") as panel_pool,
            tc.tile_pool(name="ctx", bufs=4, space="PSUM") as ctx_pool,
            tc.tile_pool(name="sums", bufs=1, space="PSUM") as sum_pool,
        ):
            masks = const_pool.tile([P, 1024], F16, tag="mask")
            nc.sync.dma_start(out=masks[:], in_=mask[:])
            ones = const_pool.tile([P, 1], F16, tag="ones")
            nc.any.memset(ones[:], 1.0)

            kT = persist.tile([P, EB, N], F16, tag="kT")
            vdram = vdram_pool.tile([P, MT, D], F16, tag="vdram")

            def q_phase(wq_tile, half):
                """Project one half (1024 local rows) of qT.

                Two e-blocks at a time fan out over 4 PSUM banks (borrowed
                from the ctx pool, idle during projections) so consecutive
                matmuls never hit the same bank (same-bank accumulation
                serializes the PE at ~+60ns/MM).
                """
                qT = persist.tile([P, EB, NQ // 2], F16, tag="qT")
                xq = stream.tile([P, DC, 1024], F16, tag="x", name=f"xq{half}")
                nc.sync.dma_start(
                    out=xq[:], in_=xq_r[:, :, half * 1024 : (half + 1) * 1024]
                )
                for ebp in range(EB // 2):
                    pss = [
                        ctx_pool.tile([P, 512], F32, tag="ctx", name=f"qp{c}")
                        for c in range(4)
                    ]  # c = 2*e + nqc for e in (0,1)
                    for dc in range(DC):
                        for e in range(2):
                            eb = 2 * ebp + e
                            lhsT = wq_tile[:, dc, eb * P : (eb + 1) * P]
                            for nqc in range(2):
                                nc.tensor.matmul(
                                    pss[2 * e + nqc][:],
                                    lhsT,
                                    xq[:, dc, nqc * 512 : (nqc + 1) * 512],
                                    start=(dc == 0),
                                    stop=(dc == DC - 1),
                                )
                    for e in range(2):
                        eb = 2 * ebp + e
                        for nqc in range(2):
                            nc.scalar.copy(
                                qT[:, eb, nqc * 512 : (nqc + 1) * 512],
                                pss[2 * e + nqc][:],
                            )
                return qT

            def kv_phase(wk_tile, wv_tile):
                """kT and V projections, streaming x_t in 2048-col halves.

                Each stationary operand fans out over 4 PSUM banks (borrowed
                from the ctx pool, idle during projections) — same-bank
                accumulation chains serialize the PE. V is spilled to DRAM
                and streamed back during attention (frees 64KB/partition of
                SBUF for larger x chunks).
                """
                for mh in range(2):
                    xt = stream.tile([P, DC, 2048], F16, tag="x", name=f"xt{mh}")
                    nc.sync.dma_start(
                        out=xt[:], in_=x_r[:, :, mh * 2048 : (mh + 1) * 2048]
                    )
                    for eb in range(EB):
                        pss = [
                            ctx_pool.tile([P, 512], F32, tag="ctx", name=f"kp{c}")
                            for c in range(4)
                        ]
                        for dc in range(DC):
                            lhsT = wk_tile[:, dc, eb * P : (eb + 1) * P]
                            for mc in range(4):
                                nc.tensor.matmul(
                                    pss[mc][:],
                                    lhsT,
                                    xt[:, dc, mc * 512 : (mc + 1) * 512],
                                    start=(dc == 0),
                                    stop=(dc == DC - 1),
                                )
                        for mc in range(4):
                            col0 = mh * 2048 + mc * 512
                            nc.scalar.copy(kT[:, eb, col0 : col0 + 512], pss[mc][:])
                    for ip in range(8):  # pairs of m blocks within this half
                        pss = [
                            ctx_pool.tile([P, 512], F32, tag="ctx", name=f"vp{c}")
                            for c in range(4)
                        ]  # c = 2*i + eh
                        for dc in range(DC):
                            for i2 in range(2):
                                i = 2 * ip + i2
                                lhsT = xt[:, dc, i * P : (i + 1) * P]
                                for eh in range(2):
                                    nc.tensor.matmul(
                                        pss[2 * i2 + eh][:],
                                        lhsT,
                                        wv_tile[:, dc, eh * 512 : (eh + 1) * 512],
                                        start=(dc == 0),
                                        stop=(dc == DC - 1),
                                    )
                        for i2 in range(2):
                            mb = mh * 16 + 2 * ip + i2
                            vs = vstream.tile([P, D], F16, tag="vout", bufs=3)
                            for eh in range(2):
                                nc.scalar.copy(
                                    vs[:, eh * 512 : (eh + 1) * 512],
                                    pss[2 * i2 + eh][:],
                                )
                            nc.sync.dma_start(out=vdram[:, mb, :], in_=vs[:])

            def attn_block(qT, j):
                """Attention for nq block j (256 local rows)."""
                jj = j % 4  # index within the qT half
                ntiles = 4 * j + 4
                ctx = [
                    [
                        ctx_pool.tile([P, 512], F32, tag="ctx", name=f"ctx{su}{eh}")
                        for eh in range(2)
                    ]
                    for su in range(2)
                ]  # [su][eh]
                sums = sum_pool.tile([P, 2], F32, tag="sums", name=f"sums{j}")

                pending = []  # (probs tile, V tile, t), consumed 2 tiles late

                def consume(pt, vt, t):
                    first = t == 0
                    last = t == ntiles - 1
                    for su in range(2):
                        lhsT = pt[:, su * P : (su + 1) * P]
                        # su=0's start=True zeroes the whole bank (both
                        # columns); su=1 must not re-start or it would wipe
                        # su=0's accumulator. Its first write lands on
                        # cleared has_written bits, which overwrites.
                        nc.tensor.matmul(
                            sums[:, su : su + 1], lhsT, ones[:],
                            start=(first and su == 0), stop=last,
                            skip_group_check=True,
                        )
                        if phases == "noctx":
                            continue
                        for eh in range(2):
                            nc.tensor.matmul(
                                ctx[su][eh][:],
                                lhsT,
                                vt[:, eh * 512 : (eh + 1) * 512],
                                start=first,
                                stop=last,
                            )

                # scores are computed for pairs of m-tiles so consecutive
                # matmuls alternate PSUM banks (same-bank chains serialize)
                for u in range(ntiles // 2):
                    pss = [
                        panel_pool.tile([P, 256], F32, tag="panel", name=f"sc{c}")
                        for c in range(2)
                    ]
                    for eb in range(EB):
                        for c in range(2):
                            t = 2 * u + c
                            nc.tensor.matmul(
                                pss[c][:],
                                kT[:, eb, t * P : (t + 1) * P],
                                qT[:, eb, jj * 256 : (jj + 1) * 256],
                                start=(eb == 0),
                                stop=(eb == EB - 1),
                            )
                    for c in range(2):
                        t = 2 * u + c
                        et = exp_pool.tile([P, 256], F16, tag="et")
                        nc.scalar.activation(
                            et[:], pss[c][:], mybir.ActivationFunctionType.Exp
                        )
                        s = t - 4 * j
                        if s >= 0:  # diagonal tile: zero out masked entries
                            me = exp_pool.tile([P, 256], F16, tag="et")
                            nc.vector.tensor_mul(
                                me[:], et[:], masks[:, s * 256 : (s + 1) * 256]
                            )
                            pt = me
                        else:
                            pt = et
                        vt = vstream.tile([P, D], F16, tag="vin", bufs=5)
                        nc.sync.dma_start(out=vt[:], in_=vdram[:, t, :])
                        if len(pending) == 2:
                            consume(*pending.pop(0))
                        pending.append((pt, vt, t))
                for args in pending:
                    consume(*args)

                recip = small_pool.tile([P, 2], F32, tag="recip")
                nc.vector.reciprocal(recip[:], sums[:])
                for su in range(2):
                    r0 = j * 256 + su * P
                    if phases == "noctx":
                        nc.sync.dma_start(
                            out=out[r0 : r0 + P, 0:1], in_=recip[:, su : su + 1]
                        )
                        continue
                    ob = out_pool.tile([P, D], F32, tag="ob")
                    for eh in range(2):
                        nc.vector.tensor_scalar_mul(
                            ob[:, eh * 512 : (eh + 1) * 512],
                            ctx[su][eh][:],
                            recip[:, su : su + 1],
                        )
                    nc.sync.dma_start(out=out[r0 : r0 + P, :], in_=ob[:])

            # ---- phase emission ----
            def emit_body():
                wq = w_pool.tile([P, DC, D], F16, tag="w", name="wq")
                nc.sync.dma_start(out=wq[:], in_=wq_r[:])
                wk = w_pool.tile([P, DC, D], F16, tag="w", name="wk")
                nc.sync.dma_start(out=wk[:], in_=wk_r[:])

                qTA = q_phase(wq, half=0)

                wv = w_pool.tile([P, DC, D], F16, tag="w", name="wv")
                nc.sync.dma_start(out=wv[:], in_=wv_r[:])
                kv_phase(wk, wv)

                if phases == "proj":
                    # keep the projection outputs live with token reads
                    ob = out_pool.tile([P, D], F32, tag="ob")
                    nc.scalar.copy(ob[:, 0:512], kT[:, 0, 0:512])
                    vs = vstream.tile([P, D], F16, tag="vin")
                    nc.sync.dma_start(out=vs[:], in_=vdram[:, 0, :])
                    nc.scalar.copy(ob[:, 512:1024], vs[:, 0:512])
                    nc.scalar.copy(ob[:, 0:512], qTA[:, 0, 0:512])
                    nc.sync.dma_start(out=out[0:P, :], in_=ob[:])
                    return

                for j in range(4):
                    attn_block(qTA, j)
                wq2 = w_pool.tile([P, DC, D], F16, tag="w", name="wq2")
                nc.sync.dma_start(out=wq2[:], in_=wq_r[:])
                qTB = q_phase(wq2, half=1)
                for j in range(4, NB):
                    attn_block(qTB, j)

            if n_reps == 1:
                emit_body()
            else:
                with tc.For_i(0, n_reps, 1):
                    emit_body()

    nc.compile()
    return nc


class Runner:
    """Compile once, keep the jitted sharded executable + static inputs on
    device, and rotate donated output buffers across calls.

    Mirrors bass2jax.run_bass_via_pjrt but caches everything reusable.
    `replicated` names inputs identical across cores (shipped once).
    """

    def __init__(self, nc, n_cores=8, replicated=()):
        import jax
        from jax.sharding import Mesh, PartitionSpec, NamedSharding
        from jax.experimental.shard_map import shard_map
        from concourse import bass2jax

        bass2jax.install_neuronx_cc_hook()
        self.jax = jax
        self.nc = nc
        self.n_cores = n_cores
        self.replicated = set(replicated)

        partition_name = (
            nc.partition_id_tensor.name if nc.partition_id_tensor else None
        )
        in_names, out_names, out_avals, zero_outs = [], [], [], []
        for alloc in nc.m.functions[0].allocations:
            if not isinstance(alloc, mybir.MemoryLocationSet):
                continue
            name = alloc.memorylocations[0].name
            if alloc.kind == "ExternalInput":
                if name != partition_name:
                    in_names.append(name)
            elif alloc.kind == "ExternalOutput":
                out_names.append(name)
                shape = tuple(alloc.tensor_shape)
                dtype = mybir.dt.np(alloc.dtype)
                out_avals.append(jax.core.ShapedArray(shape, dtype))
                zero_outs.append(np.zeros((n_cores * shape[0], *shape[1:]), dtype))
        self.in_names, self.out_names, self.out_avals = in_names, out_names, out_avals

        n_params = len(in_names)
        all_names = in_names + out_names
        if partition_name is not None:
            all_names = all_names + [partition_name]
        donate = tuple(range(n_params, n_params + len(out_names)))

        def _body(*args):
            operands = list(args)
            if partition_name is not None:
                operands.append(bass2jax.partition_id_tensor())
            outs = bass2jax._bass_exec_p.bind(
                *operands,
                out_avals=tuple(out_avals),
                in_names=tuple(all_names),
                out_names=tuple(out_names),
                lowering_input_output_aliases=(),
                sim_require_finite=True,
                sim_require_nnan=True,
                nc=nc,
            )
            return tuple(outs)

        devices = jax.devices()[:n_cores]
        self.mesh = Mesh(np.asarray(devices), ("core",))
        self.spec_sharded = NamedSharding(self.mesh, PartitionSpec("core"))
        self.spec_repl = NamedSharding(self.mesh, PartitionSpec())
        in_specs = tuple(
            PartitionSpec() if nm in self.replicated else PartitionSpec("core")
            for nm in in_names
        ) + (PartitionSpec("core"),) * len(out_names)
        out_specs = (PartitionSpec("core"),) * len(out_names)
        self.sharded = jax.jit(
            shard_map(
                _body,
                mesh=self.mesh,
                in_specs=in_specs,
                out_specs=out_specs,
                check_rep=False,
            ),
            donate_argnums=donate,
            keep_unused=True,
        )
        self._spare_outs = zero_outs  # consumed (donated) on first call
        self._static = {}  # name -> device array for cached static inputs

    def put_static(self, name, value):
        """Device-put an input once; reused for every subsequent call."""
        spec = self.spec_repl if name in self.replicated else self.spec_sharded
        self._static[name] = self.jax.device_put(value, spec)

    def __call__(self, in_maps, fetch=True):
        jax = self.jax
        args = []
        for i, nm in enumerate(self.in_names):
            if nm in self._static:
                args.append(self._static[nm])
            elif nm in self.replicated:
                args.append(jax.device_put(in_maps[0][nm], self.spec_repl))
            else:
                cat = np.concatenate([m[nm] for m in in_maps], axis=0)
                args.append(jax.device_put(cat, self.spec_sharded))
        args.extend(self._spare_outs)
        outs = self.sharded(*args)
        outs = [jax.block_until_ready(o) for o in outs]
        if not fetch:
            # benchmark mode: leave results on device (D2H over the axon
            # tunnel is slow and jittery); recycle buffers for donation
            self._spare_outs = list(outs)
            return None
        # recycle result buffers as the next call's donated outputs (the
        # kernel writes every element, so stale contents are harmless)
        self._spare_outs = list(outs)
        return [
            {
                nm: np.asarray(outs[i]).reshape(
                    self.n_cores, *self.out_avals[i].shape
                )[c]
                for i, nm in enumerate(self.out_names)
            }
            for c in range(self.n_cores)
        ]


_RUNNERS = {}


def _get_runner(n_reps: int = 1, phases: str = "all"):
    key = (n_reps, phases)
    if key not in _RUNNERS:
        nc = build_nc(n_reps, phases)
        _RUNNERS[key] = Runner(nc, n_cores=8, replicated=("wq_t", "wk_t", "wv_t"))
    return _RUNNERS[key]


def make_masks():
    """mask01[p][m_l, 256*s + i] = 1 if (128*s + m_l) <= (2*i + p) else 0."""
    i = np.arange(256)
    m_l = np.arange(P)
    out = []
    for p in range(2):
        tiles = []
        for s in range(4):
            allow = (128 * s + m_l[:, None]) <= (2 * i[None, :] + p)
            tiles.append(allow.astype(np.float16))
        out.append(np.concatenate(tiles, axis=1))
    return out


def make_in_maps(x, Wq, Wk, Wv):
    scale = np.float32(1.0 / np.sqrt(D))
    wq_t = np.ascontiguousarray((Wq.T * scale).astype(np.float16))
    wk_t = np.ascontiguousarray(Wk.T.astype(np.float16))
    wv_t = np.ascontiguousarray(Wv.T.astype(np.float16))
    masks = make_masks()
    x16 = x.astype(np.float16)
    in_maps = []
    for c in range(8):
        b, p = c // 2, c % 2
        in_maps.append(
            {
                "xq_t": np.ascontiguousarray(x16[b, p::2, :].T),
                "x_t": np.ascontiguousarray(x16[b].T),
                "wq_t": wq_t,
                "wk_t": wk_t,
                "wv_t": wv_t,
                "mask": masks[p],
            }
        )
    return in_maps


def kernel(x, Wq, Wk, Wv):
    runner = _get_runner()
    in_maps = make_in_maps(x, Wq, Wk, Wv)
    results = runner(in_maps)
    out = np.empty((B, N, D), dtype=np.float32)
    for c in range(8):
        b, p = c // 2, c % 2
        out[b, p::2, :] = results[c]["out"]
    return out


# revision 41
# speedup vs baseline: 8947.5826x; 1.3267x over previous
"""Causal attention kernel for Trainium2, 8 NeuronCores.

Problem: x [4, 4096, 1024] fp32, Wq/Wk/Wv [1024, 1024] fp32.
  q = x @ Wq.T ; k = x @ Wk.T ; v = x @ Wv.T  (per batch)
  out = softmax(causal(q k^T / sqrt(1024))) @ v

Sharding: 8 cores = 4 batches x 2 parities. Core (b, p) computes output rows
{p, p+2, ...} of batch b (interleaved rows -> balanced causal work, and the
diagonal-tile masks are identical for every row-block, so one uniform SPMD
program works for all cores with masks passed as data).

Per-core dataflow (all matmuls fp16 with fp32 PSUM accumulation):
  qT[e, nq] = WqT-chunks^T x xqT         (nq = 2048 local rows)
  kT[e, m]  = WkT-chunks^T x xT          (m = 4096, SBUF resident)
  V[m, e]   = xT-chunks^T x WvT          (spilled to DRAM, streamed back)
  scoresT[m-tile, nq-blk] = kT-chunks^T x qT-chunks  (causal extent only)
  probsT = exp(scoresT) * mask01         (no max subtraction; |scores| <~ 6)
  sums[nq] = probsT^T x ones             (PE matmul, PSUM accumulated)
  ctx[nq, e] = probsT^T x V              (PSUM accumulated over m-tiles)
  out = ctx / sums

Matmul efficiency: a LDWEIGHTS is emitted per matmul and is only partially
hidden, so projection loops keep one stationary operand loaded and fan out
over several PSUM banks (measured 216 ns/MM at N=512 vs 301 ns without).
"""

import numpy as np

import concourse.bacc as bacc
import concourse.mybir as mybir
from concourse import tile

B, N, D = 4, 4096, 1024
NQ = N // 2          # local rows per core (one parity of one batch)
P = 128              # partitions
NB = NQ // 256       # nq blocks of 256 local rows (8)
DC = D // P          # d chunks (8)
EB = D // P          # e blocks (8)
MT = N // P          # m tiles of 128 (32)

F32 = mybir.dt.float32
F16 = mybir.dt.float16


def build_nc(n_reps: int = 1, phases: str = "all"):
    """Build the kernel module. n_reps>1 wraps the whole body in an on-device
    For_i loop — used only for benchmarking (wall-clock delta between rep
    counts isolates pure HW execution time from dispatch/transfer overhead).
    phases: "all" | "proj" (projections only) | "noctx" (skip context matmuls)
    — benchmark-only variants for phase attribution.
    """
    nc = bacc.Bacc(None, target_bir_lowering=False)

    xq_t = nc.declare_dram_parameter("xq_t", [D, NQ], F16, isOutput=False)
    x_t = nc.declare_dram_parameter("x_t", [D, N], F16, isOutput=False)
    wq_t = nc.declare_dram_parameter("wq_t", [D, D], F16, isOutput=False)
    wk_t = nc.declare_dram_parameter("wk_t", [D, D], F16, isOutput=False)
    wv_t = nc.declare_dram_parameter("wv_t", [D, D], F16, isOutput=False)
    mask = nc.declare_dram_parameter("mask", [P, 1024], F16, isOutput=False)
    out = nc.declare_dram_parameter("out", [NQ, D], F32, isOutput=True)

    xq_r = xq_t.rearrange("(a p) q -> p a q", p=P)
    x_r = x_t.rearrange("(a p) m -> p a m", p=P)
    wq_r = wq_t.rearrange("(a p) e -> p a e", p=P)
    wk_r = wk_t.rearrange("(a p) e -> p a e", p=P)
    wv_r = wv_t.rearrange("(a p) e -> p a e", p=P)

    with tile.TileContext(nc) as tc:
        with (
            tc.tile_pool(name="const", bufs=1) as const_pool,
            tc.tile_pool(name="w", bufs=2) as w_pool,
            tc.tile_pool(name="persist", bufs=1) as persist,
            tc.tile_pool(name="stream", bufs=2) as stream,
            tc.tile_pool(name="vstream", bufs=5) as vstream,
            tc.tile_pool(name="exp", bufs=8) as exp_pool,
            tc.tile_pool(name="outs", bufs=2) as out_pool,
            tc.tile_pool(name="small", bufs=4) as small_pool,
            tc.tile_pool(name="vdram", bufs=1, space="DRAM") as vdram_pool,
            tc.tile_pool(name="panel", bufs=3, space="PSUM") as panel_pool,
            tc.tile_pool(name="ctx", bufs=4, space="PSUM") as ctx_pool,
            tc.tile_pool(name="sums", bufs=1, space="PSUM") as sum_pool,
        ):
            masks = const_pool.tile([P, 1024], F16, tag="mask")
            nc.sync.dma_start(out=masks[:], in_=mask[:])
            ones = const_pool.tile([P, 1], F16, tag="ones")
            nc.any.memset(ones[:], 1.0)

            kT = persist.tile([P, EB, N], F16, tag="kT")
            vdram = vdram_pool.tile([P, MT, D], F16, tag="vdram")

            def q_phase(wq_tile, half):
                """Project one half (1024 local rows) of qT.

                Two e-blocks at a time fan out over 4 PSUM banks (borrowed
                from the ctx pool, idle during projections) so consecutive
                matmuls never hit the same bank (same-bank accumulation
                serializes the PE at ~+60ns/MM).
                """
                qT = persist.tile([P, EB, NQ // 2], F16, tag="qT")
                xq = stream.tile([P, DC, 1024], F16, tag="x", name=f"xq{half}")
                nc.sync.dma_start(
                    out=xq[:], in_=xq_r[:, :, half * 1024 : (half + 1) * 1024]
                )
                for ebp in range(EB // 2):
                    pss = [
                        ctx_pool.tile([P, 512], F32, tag="ctx", name=f"qp{c}")
                        for c in range(4)
                    ]  # c = 2*e + nqc for e in (0,1)
                    for dc in range(DC):
                        for e in range(2):
                            eb = 2 * ebp + e
                            lhsT = wq_tile[:, dc, eb * P : (eb + 1) * P]
                            for nqc in range(2):
                                nc.tensor.matmul(
                                    pss[2 * e + nqc][:],
                                    lhsT,
                                    xq[:, dc, nqc * 512 : (nqc + 1) * 512],
                                    start=(dc == 0),
                                    stop=(dc == DC - 1),
                                )
                    for e in range(2):
                        eb = 2 * ebp + e
                        for nqc in range(2):
                            nc.scalar.copy(
                                qT[:, eb, nqc * 512 : (nqc + 1) * 512],
                                pss[2 * e + nqc][:],
                            )
                return qT

            def kv_phase(wk_tile, wv_tile):
                """kT and V projections, streaming x_t in 2048-col halves.

                Each stationary operand fans out over 4 PSUM banks (borrowed
                from the ctx pool, idle during projections) — same-bank
                accumulation chains serialize the PE. V is spilled to DRAM
                and streamed back during attention (frees 64KB/partition of
                SBUF for larger x chunks).
                """
                for mh in range(2):
                    xt = stream.tile([P, DC, 2048], F16, tag="x", name=f"xt{mh}")
                    nc.sync.dma_start(
                        out=xt[:], in_=x_r[:, :, mh * 2048 : (mh + 1) * 2048]
                    )
                    for eb in range(EB):
                        pss = [
                            ctx_pool.tile([P, 512], F32, tag="ctx", name=f"kp{c}")
                            for c in range(4)
                        ]
                        for dc in range(DC):
                            lhsT = wk_tile[:, dc, eb * P : (eb + 1) * P]
                            for mc in range(4):
                                nc.tensor.matmul(
                                    pss[mc][:],
                                    lhsT,
                                    xt[:, dc, mc * 512 : (mc + 1) * 512],
                                    start=(dc == 0),
                                    stop=(dc == DC - 1),
                                )
                        for mc in range(4):
                            col0 = mh * 2048 + mc * 512
                            nc.scalar.copy(kT[:, eb, col0 : col0 + 512], pss[mc][:])
                    for ip in range(8):  # pairs of m blocks within this half
                        pss = [
                            ctx_pool.tile([P, 512], F32, tag="ctx", name=f"vp{c}")
                            for c in range(4)
                        ]  # c = 2*i + eh
                        for dc in range(DC):
                            for i2 in range(2):
                                i = 2 * ip + i2
                                lhsT = xt[:, dc, i * P : (i + 1) * P]
                                for eh in range(2):
                                    nc.tensor.matmul(
                                        pss[2 * i2 + eh][:],
                                        lhsT,
                                        wv_tile[:, dc, eh * 512 : (eh + 1) * 512],
                                        start=(dc == 0),
                                        stop=(dc == DC - 1),
                                    )
                        for i2 in range(2):
                            mb = mh * 16 + 2 * ip + i2
                            vs = vstream.tile([P, D], F16, tag="vout", bufs=3)
                            for eh in range(2):
                                nc.scalar.copy(
                                    vs[:, eh * 512 : (eh + 1) * 512],
                                    pss[2 * i2 + eh][:],
                                )
                            nc.sync.dma_start(out=vdram[:, mb, :], in_=vs[:])

            def attn_block(qT, j):
                """Attention for nq block j (256 local rows)."""
                jj = j % 4  # index within the qT half
                ntiles = 4 * j + 4
                ctx = [
                    [
                        ctx_pool.tile([P, 512], F32, tag="ctx", name=f"ctx{su}{eh}")
                        for eh in range(2)
                    ]
                    for su in range(2)
                ]  # [su][eh]
                sums = sum_pool.tile([P, 2], F32, tag="sums", name=f"sums{j}")

                pending = []  # (probs tile, V tile, t), consumed 2 tiles late

                def consume(pt, vt, t):
                    first = t == 0
                    last = t == ntiles - 1
                    for su in range(2):
                        lhsT = pt[:, su * P : (su + 1) * P]
                        # su=0's start=True zeroes the whole bank (both
                        # columns); su=1 must not re-start or it would wipe
                        # su=0's accumulator. Its first write lands on
                        # cleared has_written bits, which overwrites.
                        nc.tensor.matmul(
                            sums[:, su : su + 1], lhsT, ones[:],
                            start=(first and su == 0), stop=last,
                            skip_group_check=True,
                        )
                        if phases == "noctx":
                            continue
                        for eh in range(2):
                            nc.tensor.matmul(
                                ctx[su][eh][:],
                                lhsT,
                                vt[:, eh * 512 : (eh + 1) * 512],
                                start=first,
                                stop=last,
                            )

                # scores are computed for pairs of m-tiles so consecutive
                # matmuls alternate PSUM banks (same-bank chains serialize)
                for u in range(ntiles // 2):
                    pss = [
                        panel_pool.tile([P, 256], F32, tag="panel", name=f"sc{c}")
                        for c in range(2)
                    ]
                    for eb in range(EB):
                        for c in range(2):
                            t = 2 * u + c
                            nc.tensor.matmul(
                                pss[c][:],
                                kT[:, eb, t * P : (t + 1) * P],
                                qT[:, eb, jj * 256 : (jj + 1) * 256],
                                start=(eb == 0),
                                stop=(eb == EB - 1),
                            )
                    for c in range(2):
                        t = 2 * u + c
                        et = exp_pool.tile([P, 256], F16, tag="et")
                        nc.scalar.activation(
                            et[:], pss[c][:], mybir.ActivationFunctionType.Exp
                        )
                        s = t - 4 * j
                        if s >= 0:  # diagonal tile: zero out masked entries
                            me = exp_pool.tile([P, 256], F16, tag="et")
                            nc.vector.tensor_mul(
                                me[:], et[:], masks[:, s * 256 : (s + 1) * 256]
                            )
                            pt = me
                        else:
                            pt = et
                        vt = vstream.tile([P, D], F16, tag="vin", bufs=5)
                        nc.sync.dma_start(out=vt[:], in_=vdram[:, t, :])
                        if len(pending) == 2:
                            consume(*pending.pop(0))
                        pending.append((pt, vt, t))
                for args in pending:
                    consume(*args)

                recip = small_pool.tile([P, 2], F32, tag="recip")
                nc.vector.reciprocal(recip[:], sums[:])
                for su in range(2):
                    r0 = j * 256 + su * P
                    if phases == "noctx":
                        nc.sync.dma_start(
                            out=out[r0 : r0 + P, 0:1], in_=recip[:, su : su + 1]
                        )
                        continue
                    ob = out_pool.tile([P, D], F32, tag="ob")
                    for eh in range(2):
                        nc.vector.tensor_scalar_mul(
                            ob[:, eh * 512 : (eh + 1) * 512],
                            ctx[su][eh][:],
                            recip[:, su : su + 1],
                        )
                    nc.sync.dma_start(out=out[r0 : r0 + P, :], in_=ob[:])

            # ---- phase emission ----
            def emit_body():
                wq = w_pool.tile([P, DC, D], F16, tag="w", name="wq")
                nc.sync.dma_start(out=wq[:], in_=wq_r[:])
                wk = w_pool.tile([P, DC, D], F16, tag="w", name="wk")
                nc.sync.dma_start(out=wk[:], in_=wk_r[:])

                qTA = q_phase(wq, half=0)

                wv = w_pool.tile([P, DC, D], F16, tag="w", name="wv")
                nc.sync.dma_start(out=wv[:], in_=wv_r[:])
                kv_phase(wk, wv)

                if phases == "proj":
                    # keep the projection outputs live with token reads
                    ob = out_pool.tile([P, D], F32, tag="ob")
                    nc.scalar.copy(ob[:, 0:512], kT[:, 0, 0:512])
                    vs = vstream.tile([P, D], F16, tag="vin")
                    nc.sync.dma_start(out=vs[:], in_=vdram[:, 0, :])
                    nc.scalar.copy(ob[:, 512:1024], vs[:, 0:512])
                    nc.scalar.copy(ob[:, 0:512], qTA[:, 0, 0:512])
                    nc.sync.dma_start(out=out[0:P, :], in_=ob[:])
                    return

                for j in range(4):
                    attn_block(qTA, j)
                wq2 = w_pool.tile([P, DC, D], F16, tag="w", name="wq2")
                nc.sync.dma_start(out=wq2[:], in_=wq_r[:])
                qTB = q_phase(wq2, half=1)
                for j in range(4, NB):
                    attn_block(qTB, j)

            if n_reps == 1:
                emit_body()
            else:
                with tc.For_i(0, n_reps, 1):
                    emit_body()

    nc.compile()
    return nc


class Runner:
    """Compile once, keep the jitted sharded executable + static inputs on
    device, and rotate donated output buffers across calls.

    Mirrors bass2jax.run_bass_via_pjrt but caches everything reusable.
    `replicated` names inputs identical across cores (shipped once).
    """

    def __init__(self, nc, n_cores=8, replicated=()):
        import jax
        from jax.sharding import Mesh, PartitionSpec, NamedSharding
        from jax.experimental.shard_map import shard_map
        from concourse import bass2jax

        bass2jax.install_neuronx_cc_hook()
        self.jax = jax
        self.nc = nc
        self.n_cores = n_cores
        self.replicated = set(replicated)

        partition_name = (
            nc.partition_id_tensor.name if nc.partition_id_tensor else None
        )
        in_names, out_names, out_avals, zero_outs = [], [], [], []
        for alloc in nc.m.functions[0].allocations:
            if not isinstance(alloc, mybir.MemoryLocationSet):
                continue
            name = alloc.memorylocations[0].name
            if alloc.kind == "ExternalInput":
                if name != partition_name:
                    in_names.append(name)
            elif alloc.kind == "ExternalOutput":
                out_names.append(name)
                shape = tuple(alloc.tensor_shape)
                dtype = mybir.dt.np(alloc.dtype)
                out_avals.append(jax.core.ShapedArray(shape, dtype))
                zero_outs.append(np.zeros((n_cores * shape[0], *shape[1:]), dtype))
        self.in_names, self.out_names, self.out_avals = in_names, out_names, out_avals

        n_params = len(in_names)
        all_names = in_names + out_names
        if partition_name is not None:
            all_names = all_names + [partition_name]
        donate = tuple(range(n_params, n_params + len(out_names)))

        def _body(*args):
            operands = list(args)
            if partition_name is not None:
                operands.append(bass2jax.partition_id_tensor())
            outs = bass2jax._bass_exec_p.bind(
                *operands,
                out_avals=tuple(out_avals),
                in_names=tuple(all_names),
                out_names=tuple(out_names),
                lowering_input_output_aliases=(),
                sim_require_finite=True,
                sim_require_nnan=True,
                nc=nc,
            )
            return tuple(outs)

        devices = jax.devices()[:n_cores]
        self.mesh = Mesh(np.asarray(devices), ("core",))
        self.spec_sharded = NamedSharding(self.mesh, PartitionSpec("core"))
        self.spec_repl = NamedSharding(self.mesh, PartitionSpec())
        in_specs = tuple(
            PartitionSpec() if nm in self.replicated else PartitionSpec("core")
            for nm in in_names
        ) + (PartitionSpec("core"),) * len(out_names)
        out_specs = (PartitionSpec("core"),) * len(out_names)
        self.sharded = jax.jit(
            shard_map(
                _body,
                mesh=self.mesh,
                in_specs=in_specs,
                out_specs=out_specs,
                check_rep=False,
            ),
            donate_argnums=donate,
            keep_unused=True,
        )
        self._spare_outs = zero_outs  # consumed (donated) on first call
        self._static = {}  # name -> device array for cached static inputs

    def put_static(self, name, value):
        """Device-put an input once; reused for every subsequent call."""
        spec = self.spec_repl if name in self.replicated else self.spec_sharded
        self._static[name] = self.jax.device_put(value, spec)

    def __call__(self, in_maps, fetch=True):
        jax = self.jax
        args = []
        for i, nm in enumerate(self.in_names):
            if nm in self._static:
                args.append(self._static[nm])
            elif nm in self.replicated:
                args.append(jax.device_put(in_maps[0][nm], self.spec_repl))
            else:
                cat = np.concatenate([m[nm] for m in in_maps], axis=0)
                args.append(jax.device_put(cat, self.spec_sharded))
        args.extend(self._spare_outs)
        outs = self.sharded(*args)
        outs = [jax.block_until_ready(o) for o in outs]
        if not fetch:
            # benchmark mode: leave results on device (D2H over the axon
            # tunnel is slow and jittery); recycle buffers for donation
            self._spare_outs = list(outs)
            return None
        # recycle result buffers as the next call's donated outputs (the
        # kernel writes every element, so stale contents are harmless)
        self._spare_outs = list(outs)
        return [
            {
                nm: np.asarray(outs[i]).reshape(
                    self.n_cores, *self.out_avals[i].shape
                )[c]
                for i, nm in enumerate(self.out_names)
            }
            for c in range(self.n_cores)
        ]


_RUNNERS = {}


def _get_runner(n_reps: int = 1, phases: str = "all"):
    key = (n_reps, phases)
    if key not in _RUNNERS:
        nc = build_nc(n_reps, phases)
        _RUNNERS[key] = Runner(nc, n_cores=8, replicated=("wq_t", "wk_t", "wv_t"))
    return _RUNNERS[key]


def make_masks():
    """mask01[p][m_l, 256*s + i] = 1 if (128*s + m_l) <= (2*i + p) else 0."""
    i = np.arange(256)
    m_l = np.arange(P)
    out = []
    for p in range(2):
        tiles = []
        for s in range(4):
            allow = (128 * s + m_l[:, None]) <= (2 * i[None, :] + p)
            tiles.append(allow.astype(np.float16))
        out.append(np.concatenate(tiles, axis=1))
    return out


def make_in_maps(x, Wq, Wk, Wv):
    scale = np.float32(1.0 / np.sqrt(D))
    wq_t = np.ascontiguousarray((Wq.T * scale).astype(np.float16))
    wk_t = np.ascontiguousarray(Wk.T.astype(np.float16))
    wv_t = np.ascontiguousarray(Wv.T.astype(np.float16))
    masks = make_masks()
    x16 = x.astype(np.float16)
    in_maps = []
    for c in range(8):
        b, p = c // 2, c % 2
        in_maps.append(
            {
                "xq_t": np.ascontiguousarray(x16[b, p::2, :].T),
                "x_t": np.ascontiguousarray(x16[b].T),
                "wq_t": wq_t,
                "wk_t": wk_t,
                "wv_t": wv_t,
                "mask": masks[p],
            }
        )
    return in_maps


def kernel(x, Wq, Wk, Wv):
    runner = _get_runner()
    in_maps = make_in_maps(x, Wq, Wk, Wv)
    results = runner(in_maps)
    out = np.empty((B, N, D), dtype=np.float32)
    for c in range(8):
        b, p = c // 2, c % 2
        out[b, p::2, :] = results[c]["out"]
    return out
